# revision 1
# baseline (speedup 1.0000x reference)
"""Trainium2 Bass kernel for nn_Attention_58360015618558.

Strategy (8 NeuronCores, SPMD, no collectives):
  - Shard: core c -> (batch b = c//2, seq-half h = c%2). Each core computes
    output rows for its 1024 query positions of its batch element.
  - K/V are computed for the FULL sequence on both cores of a pair
    (duplicated compute, avoids any cross-core communication).
  - All matmuls in bf16 (1 cycle/row on PE vs 4 for f32), f32 PSUM accum.
  - Attention computed in "transposed score" layout: S^T[k,q] tiles so the
    P^T needed by the AV matmul comes straight out of exp() with no
    transposes; softmax denominator via ones-vector matmul accumulation.
  - Q^T/K^T produced by PE transposes after LayerNorm+RoPE in natural layout.
"""

import math
import sys

import numpy as np

sys.path.insert(0, "/opt/trn_rl_repo")

import ml_dtypes  # noqa: E402

BF16 = ml_dtypes.bfloat16

# Full-size problem config
HID, H, KV, D, CAP = 2048, 16, 8, 128, 2048
B, S, LC = 4, 2048, 256
EPS = 1e-5
NCORES = 8

FULL_CFG = dict(S=S, SQ=S // 2, HID=HID, CAP=CAP, LC=LC, H=H, KV=KV)


def _build(cfg, gate_t, ln_trivial=(True, True, True)):
    """Build the per-core Bass program. Returns compiled Bacc."""
    import concourse.bass as bass  # noqa: F401
    import concourse.mybir as mybir
    import concourse.tile as tile
    from concourse import bacc
    from concourse.masks import make_identity
    from contextlib import ExitStack

    FP = mybir.dt.float32
    BF = mybir.dt.bfloat16
    AF = mybir.ActivationFunctionType
    ALU = mybir.AluOpType
    AX = mybir.AxisListType

    S_, SQ, HID_, CAP_, LC_ = cfg["S"], cfg["SQ"], cfg["HID"], cfg["CAP"], cfg["LC"]
    H_, KV_ = cfg["H"], cfg["KV"]
    HD, KD = H_ * D, KV_ * D
    CT, CTC = HID_ // 128, CAP_ // 128   # contraction tiles for x / caption
    NQ, NK, NLC = SQ // 128, S_ // 128, LC_ // 128
    SCALE = 1.0 / math.sqrt(D)
    qtriv, ktriv, kctriv = ln_trivial

    nc = bacc.Bacc("TRN2", target_bir_lowering=False, debug=False,
                   num_devices=NCORES)

    def din(name, shape, dt=BF):
        return nc.dram_tensor(name, shape, dt, kind="ExternalInput").ap()

    xTq = din("xTq", [HID_, SQ])        # x[b].T q-half columns (per core)
    capT = din("capT", [CAP_, LC_])
    wq = din("wq", [HID_, HD])
    wk = din("wk", [HID_, KD])
    wv = din("wv", [HID_, KD])
    wkc = din("wkc", [CAP_, KD])
    wvc = din("wvc", [CAP_, KD])
    wo = din("wo", [HD, HID_])
    cosq = din("cosq", [SQ, D], FP)   # interleaved: c[s,2i]=c[s,2i+1]=cos[s,i]
    sinq = din("sinq", [SQ, D], FP)   # interleaved signed: -sin on even, +sin on odd
    lnw = {}
    for nm, dflat in (("q", HD), ("k", KD), ("kc", KD)):
        lnw[nm] = (din(f"ln_{nm}_w", [dflat], FP), din(f"ln_{nm}_b", [dflat], FP))
    out = nc.dram_tensor("out", [SQ, HID_], FP, kind="ExternalOutput").ap()

    with ExitStack() as top:
        tc = top.enter_context(tile.TileContext(nc))

        constp = top.enter_context(tc.tile_pool(name="const", bufs=1))
        resp = top.enter_context(tc.tile_pool(name="res", bufs=1))
        dramp = top.enter_context(tc.tile_pool(name="dram", bufs=1, space="DRAM"))

        ident = constp.tile([128, 128], BF, tag="ident", name="ident")
        make_identity(nc, ident[:])
        ones_kk = constp.tile([128, 128], BF, tag="ones_kk", name="ones_kk")
        nc.vector.memset(ones_kk[:], 1.0)
        zero_c = constp.tile([128, 1], FP, tag="zero_c", name="zero_c")
        nc.vector.memset(zero_c[:], 0.0)
        nc.const_aps.aps[(FP, 0.0)] = zero_c[:]
        eps_c = constp.tile([128, 1], FP, tag="eps_c", name="eps_c")
        nc.vector.memset(eps_c[:], EPS)
        nc.const_aps.aps[(FP, EPS)] = eps_c[:]

        # LN affine params (only loaded when nontrivial)
        affs = {}
        for nm, dflat, triv in (("q", HD, qtriv), ("k", KD, ktriv),
                                ("kc", KD, kctriv)):
            if not triv:
                wsb = constp.tile([128, dflat // 128], FP, tag=f"aw_{nm}", name=f"aw_{nm}")
                bsb = constp.tile([128, dflat // 128], FP, tag=f"ab_{nm}", name=f"ab_{nm}")
                nc.sync.dma_start(wsb[:], lnw[nm][0].rearrange("(o p) -> p o", p=128))
                nc.sync.dma_start(bsb[:], lnw[nm][1].rearrange("(o p) -> p o", p=128))
                affs[nm] = (wsb, bsb)

        # Resident V (natural [k, kv*d] layout) for self and caption attention
        V_res = resp.tile([128, NK, KD], BF, tag="V_res", name="V_res")
        Vc_res = resp.tile([128, NLC, KD], BF, tag="Vc_res", name="Vc_res")

        # DRAM intermediates
        KVD = KV_ * 128
        QT = dramp.tile([H_, 128, SQ], BF, tag="QT", name="QT")
        KT_loc = dramp.tile([KVD, SQ], BF, tag="KT_loc", name="KT_loc")
        KT_g = dramp.tile([2 * KVD, SQ], BF, tag="KT_g", name="KT_g")
        V_loc = dramp.tile([SQ, KD], BF, tag="V_loc", name="V_loc")
        V_g = dramp.tile([2 * SQ, KD], BF, tag="V_g", name="V_g")
        KcT = dramp.tile([KV_, 128, LC_], BF, tag="KcT", name="KcT")
        aT = dramp.tile([H_, 128, SQ], BF, tag="aT", name="aT")

        # ---------------- projection pass helper ----------------
        def proj(ctx, xt_res, n_ct, w_dram, dflat, n_st, ln, divisor, rope,
                 cos_d, sin_d, tgt, aff):
            """One projection: out_writer(st) handles the epilogue.
            xt_res: SBUF [128, n_ct, n_st*128] (lhsT tiles)
            tgt: ("transpose", dram_tile) -> LN(+RoPE)+transpose into [h,128,s]
                 ("copy", res_tile)       -> plain cast copy into [128, st, dflat]
            """
            W = min(512, dflat)
            NCH = dflat // W
            ps_bufs = 2 if NCH <= 2 else 1
            psp = ctx.enter_context(tc.tile_pool(name="pjps", bufs=ps_bufs,
                                                 space="PSUM"))
            tpp = ctx.enter_context(tc.tile_pool(name="tp", bufs=2, space="PSUM"))
            sc = ctx.enter_context(tc.tile_pool(name="pjsc", bufs=2))

            if isinstance(w_dram, tuple):      # preloaded SBUF weight tile
                w_res = w_dram[0]
            else:
                wp = ctx.enter_context(tc.tile_pool(name="w", bufs=1))
                w_res = wp.tile([128, n_ct, dflat], BF, tag="w", name="w")
                w_view = w_dram.rearrange("(co p) d -> p co d", p=128)
                for ct in range(n_ct):
                    nc.sync.dma_start(w_res[:, ct, :], w_view[:, ct, :])

            def emit_transposes(roped_t, st_t):
                nh_t = dflat // 128
                tmode, tdram = tgt
                for hh in range(nh_t):
                    pst = tpp.tile([128, 128], BF, tag="tp", name="tp")
                    nc.tensor.transpose(pst[:],
                                        roped_t[:, hh * 128:(hh + 1) * 128],
                                        ident[:])
                    stg = sc.tile([128, 128], BF, tag="stg", name="stg")
                    if aff is None:
                        nc.scalar.copy(stg[:], pst[:])
                    else:
                        wsb, bsb = aff
                        nc.vector.tensor_scalar(stg[:], pst[:],
                                                wsb[:, hh:hh + 1],
                                                bsb[:, hh:hh + 1],
                                                ALU.mult, ALU.add)
                    if tmode == "t_dram3":
                        nc.sync.dma_start(
                            tdram[hh, :, st_t * 128:(st_t + 1) * 128], stg[:])
                    else:  # t_rows
                        nc.sync.dma_start(
                            tdram[hh * 128:(hh + 1) * 128,
                                  st_t * 128:(st_t + 1) * 128], stg[:])

            pending = None
            for st in range(n_st):
                ps = [psp.tile([128, W], FP, tag=f"c{i}", name=f"c{i}",
                               bufs=(2 if (NCH == 4 and i < 2) else None))
                      for i in range(NCH)]
                for i in range(NCH):
                    for ct in range(n_ct):
                        nc.tensor.matmul(
                            ps[i][:],
                            lhsT=xt_res[:, ct, st * 128:(st + 1) * 128],
                            rhs=w_res[:, ct, i * W:(i + 1) * W],
                            start=(ct == 0), stop=(ct == n_ct - 1),
                        )
                if ln and pending is not None:
                    emit_transposes(*pending)
                    pending = None
                if not ln:
                    mode, dst = tgt
                    for i in range(NCH):
                        if mode == "copy_res":
                            nc.scalar.copy(dst[:, st, i * W:(i + 1) * W], ps[i][:])
                        else:  # dma_rows: stage cast then DMA to DRAM rows
                            vstg = sc.tile([128, W], BF, tag="vstg", name="vstg")
                            nc.scalar.copy(vstg[:], ps[i][:])
                            nc.sync.dma_start(
                                dst[st * 128:(st + 1) * 128, i * W:(i + 1) * W],
                                vstg[:])
                    continue

                # LayerNorm stats over the flat dflat dim
                stats = sc.tile([128, 16], FP, tag="stats", name="stats")
                sqj = sc.tile([128, W], FP, tag="sqj", name="sqj")
                for i in range(NCH):
                    nc.vector.reduce_sum(stats[:, i:i + 1], ps[i][:], axis=AX.X)
                    nc.scalar.activation(sqj[:], ps[i][:], AF.Square,
                                         accum_out=stats[:, 8 + i:9 + i])
                # combine partials
                def fold(base):
                    npart = NCH
                    width = 1
                    while width < npart:
                        for i in range(0, npart, 2 * width):
                            if i + width < npart:
                                nc.vector.tensor_tensor(
                                    stats[:, base + i:base + i + 1],
                                    stats[:, base + i:base + i + 1],
                                    stats[:, base + i + width:base + i + width + 1],
                                    ALU.add)
                        width *= 2
                fold(0)
                fold(8)
                m = stats[:, 14:15]
                rs = stats[:, 15:16]
                nc.vector.tensor_scalar_mul(m, stats[:, 0:1], 1.0 / divisor)
                nc.vector.tensor_scalar_mul(stats[:, 9:10], stats[:, 8:9],
                                            1.0 / divisor)
                nc.vector.tensor_tensor(stats[:, 10:11], m, m, ALU.mult)
                nc.vector.tensor_tensor(stats[:, 11:12], stats[:, 9:10],
                                        stats[:, 10:11], ALU.subtract)
                nc.scalar.activation(stats[:, 12:13], stats[:, 11:12], AF.Sqrt,
                                     bias=EPS)
                nc.vector.reciprocal(rs, stats[:, 12:13])

                norm = sc.tile([128, dflat], BF, tag="norm", name="norm")
                for i in range(NCH):
                    nc.vector.tensor_scalar(norm[:, i * W:(i + 1) * W], ps[i][:],
                                            m, rs, ALU.subtract, ALU.mult)

                nh = dflat // 128
                roped = sc.tile([128, dflat], BF, tag="roped", name="roped")
                if rope:
                    # o = x*ci + swap_pairs(x)*si with ci/si pre-interleaved
                    # host-side (si sign-folded), so the 3 big multiplies are
                    # contiguous; only the pair-swap copies are strided.
                    cs = sc.tile([128, D], FP, tag="cs", name="cs")
                    sn = sc.tile([128, D], FP, tag="sn", name="sn")
                    nc.sync.dma_start(cs[:], cos_d[st * 128:(st + 1) * 128, :])
                    nc.sync.dma_start(sn[:], sin_d[st * 128:(st + 1) * 128, :])
                    cib = cs[:, None, :].to_broadcast([128, nh, D])
                    sib = sn[:, None, :].to_broadcast([128, nh, D])
                    nv = norm.rearrange("p (h d) -> p h d", h=nh)
                    n2 = norm.rearrange("p (h i two) -> p h i two", two=2, h=nh)
                    sh = sc.tile([128, nh, D], BF, tag="sh", name="sh")
                    s2 = sh.rearrange("p h (i two) -> p h i two", two=2)
                    nc.vector.tensor_copy(s2[:, :, :, 0], n2[:, :, :, 1])
                    nc.vector.tensor_copy(s2[:, :, :, 1], n2[:, :, :, 0])
                    t0 = sc.tile([128, nh, D], BF, tag="ro0", name="ro0")
                    t1 = sc.tile([128, nh, D], BF, tag="ro1", name="ro1")
                    nc.vector.tensor_tensor(t0[:], nv, cib, ALU.mult)
                    nc.vector.tensor_tensor(t1[:], sh[:], sib, ALU.mult)
                    nc.vector.tensor_tensor(
                        roped.rearrange("p (h d) -> p h d", h=nh), t0[:], t1[:],
                        ALU.add)
                else:
                    nc.scalar.copy(roped[:], norm[:])

                pending = (roped, st)
            if pending is not None:
                emit_transposes(*pending)

        # ---------------- Phase A: projections ----------------
        # K/V are computed only for this core's seq-half, then all-gathered
        # across the (even, odd) core pair sharing the batch element.
        RG = [[2 * i, 2 * i + 1] for i in range(NCORES // 2)]
        with ExitStack() as pa:
            xtp = pa.enter_context(tc.tile_pool(name="xtq", bufs=1))
            wqp = pa.enter_context(tc.tile_pool(name="wqp", bufs=1))
            xtq_res = xtp.tile([128, CT, SQ], BF, tag="xtq", name="xtq")
            xtq_view = xTq.rearrange("(co p) s -> p co s", p=128)
            wq_res = wqp.tile([128, CT, HD], BF, tag="wq", name="wq")
            wq_view = wq.rearrange("(co p) d -> p co d", p=128)
            with ExitStack() as pv:
                wvp = pv.enter_context(tc.tile_pool(name="wvp", bufs=1))
                wv_res = wvp.tile([128, CT, KD], BF, tag="wv", name="wv")
                wv_view = wv.rearrange("(co p) d -> p co d", p=128)
                for ct in range(CT):
                    nc.sync.dma_start(wv_res[:, ct, :], wv_view[:, ct, :])
                    nc.sync.dma_start(xtq_res[:, ct, :], xtq_view[:, ct, :])
                with ExitStack() as ph:
                    proj(ph, xtq_res, CT, (wv_res,), KD, NQ, ln=False,
                         divisor=None, rope=False, cos_d=None, sin_d=None,
                         tgt=("dma_rows", V_loc), aff=None)
            nc.gpsimd.collective_compute(
                "AllGather", ALU.bypass, replica_groups=RG,
                ins=[V_loc.opt()], outs=[V_g.opt()])
            # fills depend on the collective: keep them off the Sync queue
            for b2 in range(2):
                for stl in range(NQ):
                    nc.gpsimd.dma_start(
                        V_res[:, b2 * NQ + stl, :],
                        V_g[b2 * SQ + stl * 128:b2 * SQ + (stl + 1) * 128, :])
            with ExitStack() as ph:
                proj(ph, xtq_res, CT, wk, KD, NQ, ln=True, divisor=KD,
                     rope=True, cos_d=cosq, sin_d=sinq,
                     tgt=("t_rows", KT_loc), aff=affs.get("k"))
            for ct in range(CT):
                nc.sync.dma_start(wq_res[:, ct, :], wq_view[:, ct, :])
            nc.gpsimd.collective_compute(
                "AllGather", ALU.bypass, replica_groups=RG,
                ins=[KT_loc.opt()], outs=[KT_g.opt()])
            with ExitStack() as ph:
                proj(ph, xtq_res, CT, (wq_res,), HD, NQ, ln=True, divisor=HD,
                     rope=True, cos_d=cosq, sin_d=sinq,
                     tgt=("t_dram3", QT), aff=affs.get("q"))
        with ExitStack() as pc2:
            ctp = pc2.enter_context(tc.tile_pool(name="ct", bufs=1))
            cap_res = ctp.tile([128, CTC, LC_], BF, tag="cap", name="cap")
            cap_view = capT.rearrange("(co p) s -> p co s", p=128)
            for ct in range(CTC):
                nc.sync.dma_start(cap_res[:, ct, :], cap_view[:, ct, :])
            with ExitStack() as ph:
                proj(ph, cap_res, CTC, wvc, KD, NLC, ln=False, divisor=None,
                     rope=False, cos_d=None, sin_d=None,
                     tgt=("copy_res", Vc_res), aff=None)
            with ExitStack() as ph:
                proj(ph, cap_res, CTC, wkc, KD, NLC, ln=True, divisor=KD,
                     rope=False, cos_d=None, sin_d=None,
                     tgt=("t_dram3", KcT), aff=affs.get("kc"))

        # ---------------- Phase B: attention ----------------
        QCH = min(512, SQ)
        NQC = SQ // QCH
        wop = top.enter_context(tc.tile_pool(name="wop", bufs=1))
        wo_res = wop.tile([128, H_, HID_], BF, tag="wo", name="wo")
        wo_view = wo.rearrange("(ho p) e -> p ho e", p=128)
        for hh in range(H_):
            nc.sync.dma_start(wo_res[:, hh, :], wo_view[:, hh, :])
        with ExitStack() as pb:
            kp = pb.enter_context(tc.tile_pool(name="kw", bufs=2))
            qp = pb.enter_context(tc.tile_pool(name="qw", bufs=2))
            ptp = pb.enter_context(tc.tile_pool(name="pt", bufs=3))
            scp = pb.enter_context(tc.tile_pool(name="sc2", bufs=2))
            aop = pb.enter_context(tc.tile_pool(name="ao", bufs=2))
            ps_s = pb.enter_context(tc.tile_pool(name="ps_s", bufs=3, space="PSUM"))
            ps_o = pb.enter_context(tc.tile_pool(name="ps_o", bufs=1, space="PSUM"))
            ps_oc = pb.enter_context(tc.tile_pool(name="ps_oc", bufs=1, space="PSUM"))
            ps_d = pb.enter_context(tc.tile_pool(name="ps_d", bufs=2, space="PSUM"))
            ps_dc = pb.enter_context(tc.tile_pool(name="ps_dc", bufs=1, space="PSUM"))

            for kv in range(KV_):
                ktw = kp.tile([128, S_], BF, tag="ktw", name="ktw")
                nc.sync.dma_start(ktw[:, 0:SQ],
                                  KT_g[kv * 128:(kv + 1) * 128, :])
                nc.sync.dma_start(ktw[:, SQ:2 * SQ],
                                  KT_g[KVD + kv * 128:KVD + (kv + 1) * 128, :])
                kcw = kp.tile([128, LC_], BF, tag="kcw", name="kcw")
                nc.sync.dma_start(kcw[:], KcT[kv])
                for rep in range(H_ // KV_):
                    h = kv * (H_ // KV_) + rep
                    qtw = qp.tile([128, SQ], BF, tag="qtw", name="qtw")
                    nc.sync.dma_start(qtw[:], QT[h])
                    for ch in range(NQC):
                        qs = qtw[:, ch * QCH:(ch + 1) * QCH]
                        po = ps_o.tile([128, QCH], FP, tag="po", name="po")
                        poc = ps_oc.tile([128, QCH], FP, tag="poc", name="poc")
                        # denominators broadcast across all 128 partitions via
                        # an all-ones stationary operand (no separate bcast)
                        pden = ps_d.tile([128, QCH], FP, tag="pden", name="pden")
                        pdenc = ps_dc.tile([128, QCH], FP, tag="pdenc",
                                           name="pdenc")
                        for kt in range(NK):
                            pscore = ps_s.tile([128, QCH], FP, tag="score",
                                               name="score")
                            nc.tensor.matmul(pscore[:],
                                             lhsT=ktw[:, kt * 128:(kt + 1) * 128],
                                             rhs=qs, start=True, stop=True)
                            pt = ptp.tile([128, QCH], BF, tag="pt", name="pt")
                            nc.scalar.activation(pt[:], pscore[:], AF.Exp,
                                                 scale=SCALE)
                            nc.tensor.matmul(
                                po[:], lhsT=V_res[:, kt, kv * 128:(kv + 1) * 128],
                                rhs=pt[:], start=(kt == 0), stop=(kt == NK - 1))
                            nc.tensor.matmul(
                                pden[:], lhsT=ones_kk[:], rhs=pt[:],
                                start=(kt == 0), stop=(kt == NK - 1))
                        for kt in range(NLC):
                            pscore = ps_s.tile([128, QCH], FP, tag="score",
                                               name="score")
                            nc.tensor.matmul(pscore[:],
                                             lhsT=kcw[:, kt * 128:(kt + 1) * 128],
                                             rhs=qs, start=True, stop=True)
                            pt = ptp.tile([128, QCH], BF, tag="pt", name="pt")
                            nc.scalar.activation(pt[:], pscore[:], AF.Exp,
                                                 scale=SCALE)
                            nc.tensor.matmul(
                                poc[:], lhsT=Vc_res[:, kt, kv * 128:(kv + 1) * 128],
                                rhs=pt[:], start=(kt == 0), stop=(kt == NLC - 1))
                            nc.tensor.matmul(
                                pdenc[:], lhsT=ones_kk[:], rhs=pt[:],
                                start=(kt == 0), stop=(kt == NLC - 1))
                        rden = scp.tile([128, QCH], FP, tag="rden", name="rden")
                        nc.vector.reciprocal_approx_fast(rden[:], pden[:])
                        t2 = scp.tile([128, QCH], FP, tag="t2", name="t2")
                        nc.vector.tensor_tensor(t2[:], po[:], rden[:], ALU.mult)
                        rdenc = scp.tile([128, QCH], FP, tag="rdenc", name="rdenc")
                        nc.vector.reciprocal_approx_fast(rdenc[:], pdenc[:])
                        tmp = scp.tile([128, QCH], FP, tag="tmp", name="tmp")
                        nc.vector.scalar_tensor_tensor(
                            tmp[:], poc[:], float(gate_t[h]), rdenc[:],
                            ALU.mult, ALU.mult)
                        ao = aop.tile([128, QCH], BF, tag="ao", name="ao")
                        nc.vector.tensor_tensor(ao[:], t2[:], tmp[:], ALU.add)
                        nc.sync.dma_start(aT[h, :, ch * QCH:(ch + 1) * QCH], ao[:])

        # ---------------- Phase C: output projection ----------------
        with ExitStack() as pc:
            ap_ = pc.enter_context(tc.tile_pool(name="ast", bufs=2))
            op_ = pc.enter_context(tc.tile_pool(name="osb", bufs=2))
            cps = pc.enter_context(tc.tile_pool(name="cps", bufs=4, space="PSUM"))
            EW = min(512, HID_)
            NEC = HID_ // EW
            for st in range(NQ):
                a_st = ap_.tile([128, H_, 128], BF, tag="ast", name="ast")
                for h in range(H_):
                    nc.sync.dma_start(a_st[:, h, :],
                                      aT[h, :, st * 128:(st + 1) * 128])
                for ec in range(NEC):
                    ps = cps.tile([128, EW], FP, tag="cps", name="cps")
                    for h in range(H_):
                        nc.tensor.matmul(ps[:], lhsT=a_st[:, h, :],
                                         rhs=wo_res[:, h, ec * EW:(ec + 1) * EW],
                                         start=(h == 0), stop=(h == H_ - 1))
                    osb = op_.tile([128, EW], FP, tag="osb", name="osb")
                    nc.scalar.copy(osb[:], ps[:])
                    nc.sync.dma_start(
                        out[st * 128:(st + 1) * 128, ec * EW:(ec + 1) * EW], osb[:])

    nc.compile()
    return nc


_CACHE = {}


def _get_program(cfg, gate_t, ln_trivial):
    key = (tuple(sorted(cfg.items())), tuple(np.round(gate_t, 8)), ln_trivial)
    if key not in _CACHE:
        _CACHE[key] = _build(cfg, gate_t, ln_trivial)
    return _CACHE[key]


def make_in_maps(cfg, inputs):
    """Host-side sharding: returns (in_maps, gate_t, ln_trivial)."""
    S_, SQ = cfg["S"], cfg["SQ"]
    x = np.asarray(inputs["x"], np.float32)
    cap = np.asarray(inputs["caption_feat"], np.float32)
    cos = np.ascontiguousarray(np.asarray(inputs["freqs_cos"], np.float32))
    sin = np.ascontiguousarray(np.asarray(inputs["freqs_sin"], np.float32))
    gate_t = np.tanh(np.asarray(inputs["gate"], np.float32))

    def bf(a):
        return np.ascontiguousarray(a).astype(BF16)

    weights = {k: bf(np.asarray(inputs[k], np.float32))
               for k in ("wq", "wk", "wv", "wo")}
    weights["wkc"] = bf(np.asarray(inputs["wk_cap"], np.float32))
    weights["wvc"] = bf(np.asarray(inputs["wv_cap"], np.float32))

    lns = {}
    triv = []
    for nm, wk_, bk_ in (("q", "q_ln_w", "q_ln_b"), ("k", "k_ln_w", "k_ln_b"),
                         ("kc", "kc_ln_w", "kc_ln_b")):
        w = np.ascontiguousarray(np.asarray(inputs[wk_], np.float32))
        b = np.ascontiguousarray(np.asarray(inputs[bk_], np.float32))
        triv.append(bool(np.all(w == 1.0) and np.all(b == 0.0)))
        lns[f"ln_{nm}_w"] = w
        lns[f"ln_{nm}_b"] = b

    in_maps = []
    for c in range(NCORES):
        b_, half = divmod(c, 2)
        xTb = bf(x[b_].T)
        m = dict(
            xTq=np.ascontiguousarray(xTb[:, half * SQ:(half + 1) * SQ]),
            capT=bf(cap[b_].T),
            cosq=np.ascontiguousarray(
                np.repeat(cos[half * SQ:(half + 1) * SQ], 2, axis=1)),
            sinq=np.ascontiguousarray(
                np.repeat(sin[half * SQ:(half + 1) * SQ], 2, axis=1)
                * np.tile([-1.0, 1.0], cos.shape[1]).astype(np.float32)),
            **weights, **lns,
        )
        in_maps.append(m)
    return in_maps, gate_t, tuple(triv)


def _install_ntff_hook():
    """Shim the missing antenv.axon_hooks module so trace=True can capture
    NTFF profiles via the axon .so (test-time only)."""
    import types

    try:
        import antenv.axon_hooks  # noqa: F401
        return
    except ImportError:
        pass
    mod = types.ModuleType("antenv.axon_hooks")
    mod._hook = None

    def set_axon_ntff_profile_hook(h):
        mod._hook = h

    def get_axon_ntff_profile_hook():
        return mod._hook

    mod.set_axon_ntff_profile_hook = set_axon_ntff_profile_hook
    mod.get_axon_ntff_profile_hook = get_axon_ntff_profile_hook
    sys.modules["antenv.axon_hooks"] = mod
    import antenv
    antenv.axon_hooks = mod
    try:
        from trn_agent_boot.trn_boot import _ntff_profile_via_ctypes
        hook = _ntff_profile_via_ctypes("/opt/axon/libaxon_pjrt.so")
        if hook is not None:
            mod._hook = hook
    except Exception as e:  # degrade to no tracing
        print("ntff hook install failed:", e, file=sys.stderr)


def run_shards(cfg, inputs, trace=False):
    """Compile (cached), run on 8 cores, return (list of per-core outs, results)."""
    from concourse import bass_utils
    if trace:
        _install_ntff_hook()
    in_maps, gate_t, triv = make_in_maps(cfg, inputs)
    nc = _get_program(cfg, gate_t, triv)
    res = bass_utils.run_bass_kernel_spmd(
        nc, in_maps, core_ids=list(range(NCORES)), trace=trace)
    return [r["out"] for r in res.results], res


def kernel(**inputs):
    outs, _ = run_shards(FULL_CFG, inputs, trace=False)
    SQ = FULL_CFG["SQ"]
    full = np.empty((B, S, HID), np.float32)
    for c in range(NCORES):
        b_, half = divmod(c, 2)
        full[b_, half * SQ:(half + 1) * SQ, :] = outs[c]
    return full



# revision 18
# speedup vs baseline: 1.1059x; 1.1059x over previous
"""Trainium2 Bass kernel for nn_Attention_58360015618558 (v2).

Strategy (8 NeuronCores, SPMD):
  - Shard: core c -> (batch b = c//2, seq-half h = c%2); each core produces
    the output rows for its 1024 query positions.
  - K/V computed for the local seq-half only, AllGathered within the
    (even, odd) core pair; collectives overlap Q/caption projections.
  - LayerNorm mean folded into host-centered weights (z - mean(z) =
    x @ w_centered); LN rsqrt fused into the RoPE cos/sin multipliers, so
    the projection epilogue is 2 DVE passes + 2 gpsimd strided passes.
  - Attention in transposed-score layout (S^T[k,q]); exp on ACT over
    [128,1024] two-bank PSUM tiles; softmax denominator via fp8e5 (e5m2)
    DoubleRow matmuls (2x PE throughput; positive sums average the fp8
    quantization error down to ~0.2%).
  - K^T, V, Kc^T, Vc, and the attention output a^T all SBUF-resident;
    wo resident during phase B; single batched DMAs with multi-dim APs
    everywhere (DMA prepare costs ~1us of engine time per trigger).
"""

import math
import sys

import numpy as np

sys.path.insert(0, "/opt/trn_rl_repo")

import ml_dtypes  # noqa: E402

BF16 = ml_dtypes.bfloat16

# Full-size problem config
HID, H, KV, D, CAP = 2048, 16, 8, 128, 2048
B, S, LC = 4, 2048, 256
EPS = 1e-5
NCORES = 8

FULL_CFG = dict(S=S, SQ=S // 2, HID=HID, CAP=CAP, LC=LC, H=H, KV=KV)

DEN_FP8 = True    # softmax denominator via fp8e5 DoubleRow matmuls
EXP_PAIR = True   # single exp over [128,1024] two-bank PSUM tiles


def _build(cfg, gate_t, ln_trivial=(True, True, True)):
    """Build the per-core Bass program. Returns compiled Bacc."""
    import concourse.bass as bass  # noqa: F401
    import concourse.mybir as mybir
    import concourse.tile as tile
    from concourse import bacc
    from concourse.masks import make_identity
    from contextlib import ExitStack

    FP = mybir.dt.float32
    BF = mybir.dt.bfloat16
    F8 = mybir.dt.float8e5
    AF = mybir.ActivationFunctionType
    ALU = mybir.AluOpType
    DR = mybir.MatmulPerfMode.DoubleRow

    S_, SQ, HID_, CAP_, LC_ = cfg["S"], cfg["SQ"], cfg["HID"], cfg["CAP"], cfg["LC"]
    H_, KV_ = cfg["H"], cfg["KV"]
    HD, KD = H_ * D, KV_ * D
    CT, CTC = HID_ // 128, CAP_ // 128   # contraction tiles for x / caption
    NQ, NK, NLC = SQ // 128, S_ // 128, LC_ // 128
    SCALE = 1.0 / math.sqrt(D)
    qtriv, ktriv, kctriv = ln_trivial

    nc = bacc.Bacc("TRN2", target_bir_lowering=False, debug=False,
                   num_devices=NCORES)

    def din(name, shape, dt=BF):
        return nc.dram_tensor(name, shape, dt, kind="ExternalInput").ap()

    xTq = din("xTq", [HID_, SQ])        # x[b].T q-half columns (per core)
    capT = din("capT", [CAP_, LC_])
    wq = din("wq", [HID_, HD])          # column-centered host-side
    wk = din("wk", [HID_, KD])          # column-centered host-side
    wv = din("wv", [HID_, KD])
    wkc = din("wkc", [CAP_, KD])        # column-centered host-side
    wvc = din("wvc", [CAP_, KD])
    wo = din("wo", [HD, HID_])
    cosq = din("cosq", [SQ, D], FP)   # interleaved: c[s,2i]=c[s,2i+1]=cos[s,i]
    sinq = din("sinq", [SQ, D], FP)   # interleaved signed: -sin even, +sin odd
    lnw = {}
    for nm, dflat in (("q", HD), ("k", KD), ("kc", KD)):
        lnw[nm] = (din(f"ln_{nm}_w", [dflat], FP), din(f"ln_{nm}_b", [dflat], FP))
    out = nc.dram_tensor("out", [SQ, HID_], FP, kind="ExternalOutput").ap()

    # SPMD: all 8 cores run one graph.  Local K/V halves are staged to
    # _loc DRAM, AllGathered within the core pair, and both halves are
    # re-filled from the gathered tensor (rank-independent).  Collective
    # completion is enforced by gpsimd queue order (fills on gpsimd).
    with ExitStack() as top:
        tc = top.enter_context(tile.TileContext(nc))

        constp = top.enter_context(tc.tile_pool(name="const", bufs=1))
        resp = top.enter_context(tc.tile_pool(name="res", bufs=1))
        dramp = top.enter_context(tc.tile_pool(name="dram", bufs=1, space="DRAM"))

        ident = constp.tile([128, 128], BF, tag="ident", name="ident")
        make_identity(nc, ident[:])
        ones8 = constp.tile([128, 2, 128], F8, tag="ones8", name="ones8")
        nc.vector.memset(ones8[:], 1.0)
        ones_bk = constp.tile([128, 128], BF, tag="ones_bk", name="ones_bk")
        nc.vector.memset(ones_bk[:], 1.0)
        zero_c = constp.tile([128, 1], FP, tag="zero_c", name="zero_c")
        nc.vector.memset(zero_c[:], 0.0)
        nc.const_aps.aps[(FP, 0.0)] = zero_c[:]
        for ci, v in enumerate({float(KD * EPS), float(HD * EPS)}):
            ec_ = constp.tile([128, 1], FP, tag=f"eps{ci}", name=f"eps{ci}")
            nc.vector.memset(ec_[:], v)
            nc.const_aps.aps[(FP, v)] = ec_[:]

        # LN affine params (only loaded when nontrivial)
        affs = {}
        for nm, dflat, triv in (("q", HD, qtriv), ("k", KD, ktriv),
                                ("kc", KD, kctriv)):
            if not triv:
                wsb = constp.tile([128, dflat // 128], FP, tag=f"aw_{nm}",
                                  name=f"aw_{nm}")
                bsb = constp.tile([128, dflat // 128], FP, tag=f"ab_{nm}",
                                  name=f"ab_{nm}")
                nc.gpsimd.dma_start(wsb[:], lnw[nm][0].rearrange(
                    "(o p) -> p o", p=128))
                nc.gpsimd.dma_start(bsb[:], lnw[nm][1].rearrange(
                    "(o p) -> p o", p=128))
                affs[nm] = (wsb, bsb)

        # SBUF residents
        KT_sb = resp.tile([128, KV_, 2, SQ], BF, tag="KT_sb", name="KT_sb")
        V_res = resp.tile([128, NK, KD], BF, tag="V_res", name="V_res")
        KcT_sb = resp.tile([128, KV_, LC_], BF, tag="KcT_sb", name="KcT_sb")
        Vc_res = resp.tile([128, NLC, KD], BF, tag="Vc_res", name="Vc_res")

        # DRAM intermediates
        KT_loc = dramp.tile([KV_, 128, SQ], BF, tag="KT_loc", name="KT_loc")
        KT_g = dramp.tile([2, KV_, 128, SQ], BF, tag="KT_g", name="KT_g")
        V_loc = dramp.tile([SQ, KD], BF, tag="V_loc", name="V_loc")
        V_g = dramp.tile([2, SQ, KD], BF, tag="V_g", name="V_g")
        QT = dramp.tile([128, H_, SQ], BF, tag="QT", name="QT")  # [d, h, s]

        # ---------------- projection pass ----------------
        def proj(ctx, xt_tiles, n_ct, w_tiles, dflat, n_st, ln, rope,
                 cos_sb, sin_sb, tgt, aff, swap_eng=None):
            """One projection with fused LN(+RoPE) epilogue.

            xt_tiles/w_tiles: list of SBUF tiles [128, g, *] covering n_ct
              contraction tiles (group size g).
            tgt: ("kt", kt_dst_fn)   hh -> AP to write [128,128] transposed
                 ("qt", None)        -> stage [128,H,128] + 1 DMA to QT
                 ("vres", slot_ap_fn) st, i, W -> AP for plain copy
            """
            W = 512
            NCH = dflat // W
            gsz = n_ct // len(xt_tiles)
            psp = ctx.enter_context(tc.tile_pool(name="pjps", bufs=1,
                                                 space="PSUM"))
            tpp = ctx.enter_context(tc.tile_pool(name="tp", bufs=2,
                                                 space="PSUM"))
            sc = ctx.enter_context(tc.tile_pool(name="pjsc", bufs=2))
            qsp = ctx.enter_context(tc.tile_pool(name="qstg", bufs=2))

            SQRTD = math.sqrt(float(dflat))

            def epilogue(ps, st):
                nh = dflat // 128
                if ln:
                    # variance of the (already mean-centered) projection
                    stats = sc.tile([128, 8], FP, tag="stats", name="stats")
                    sqj = sc.tile([128, W], FP, tag="sqj", name="sqj")
                    for i in range(NCH):
                        nc.scalar.activation(sqj[:], ps[i][:], AF.Square,
                                             accum_out=stats[:, i:i + 1])
                    width = 1
                    while width < NCH:
                        for i in range(0, NCH, 2 * width):
                            if i + width < NCH:
                                nc.vector.tensor_tensor(
                                    stats[:, i:i + 1], stats[:, i:i + 1],
                                    stats[:, i + width:i + width + 1], ALU.add)
                        width *= 2
                    nc.scalar.activation(stats[:, 6:7], stats[:, 0:1],
                                         AF.Sqrt, bias=float(dflat * EPS))
                    rs = stats[:, 7:8]
                    nc.vector.reciprocal(rs, stats[:, 6:7])

                roped = sc.tile([128, dflat], BF, tag="roped", name="roped")
                if rope:
                    # o = z*(rs*c*sqrt(d)) + swap(z)*(rs*s_signed*sqrt(d))
                    rc = sc.tile([128, D], FP, tag="rc", name="rc")
                    rsig = sc.tile([128, D], FP, tag="rsig", name="rsig")
                    nc.vector.tensor_scalar(rc[:], cos_sb[:, st, :], rs,
                                            SQRTD, ALU.mult, ALU.mult)
                    nc.vector.tensor_scalar(rsig[:], sin_sb[:, st, :], rs,
                                            SQRTD, ALU.mult, ALU.mult)
                    rce = rc[:, None, :]
                    sw = sc.tile([128, dflat], BF, tag="sw", name="sw")
                    rsig_v = rsig.rearrange("p (i two) -> p i two", two=2)
                    for i in range(NCH):
                        wh = W // D
                        rv = roped[:, i * W:(i + 1) * W].rearrange(
                            "p (h d) -> p h d", h=wh)
                        pv = ps[i].rearrange("p (h d) -> p h d", h=wh)
                        nc.vector.tensor_tensor(
                            rv, pv, rce.to_broadcast([128, wh, D]), ALU.mult)
                        p2 = ps[i].rearrange("p (h i two) -> p h i two",
                                             two=2, h=wh)
                        s2 = sw[:, i * W:(i + 1) * W].rearrange(
                            "p (h i two) -> p h i two", two=2, h=wh)
                        swap_eng.tensor_tensor(
                            s2[:, :, :, 0], p2[:, :, :, 1],
                            rsig_v[:, None, :, 0].to_broadcast(
                                [128, wh, D // 2]), ALU.mult)
                        swap_eng.tensor_tensor(
                            s2[:, :, :, 1], p2[:, :, :, 0],
                            rsig_v[:, None, :, 1].to_broadcast(
                                [128, wh, D // 2]), ALU.mult)
                        nc.vector.tensor_tensor(
                            roped[:, i * W:(i + 1) * W],
                            roped[:, i * W:(i + 1) * W],
                            sw[:, i * W:(i + 1) * W], ALU.add)
                elif ln:
                    for i in range(NCH):
                        nc.vector.tensor_scalar(
                            roped[:, i * W:(i + 1) * W], ps[i][:], rs,
                            SQRTD, ALU.mult, ALU.mult)
                else:
                    mode, dst_fn = tgt
                    for i in range(NCH):
                        nc.scalar.copy(dst_fn(st, i, W), ps[i][:])
                    return

                # transpose + write out
                mode, dst_fn = tgt
                qstg = None
                if mode == "qt":
                    qstg = qsp.tile([128, nh, 128], BF, tag="qstg",
                                    name="qstg")
                for hh in range(nh):
                    pst = tpp.tile([128, 128], BF, tag="tp", name="tp")
                    nc.tensor.transpose(pst[:],
                                        roped[:, hh * 128:(hh + 1) * 128],
                                        ident[:])
                    if mode == "qt":
                        dst = qstg[:, hh, :]
                    else:
                        dst = dst_fn(hh, st)
                    if aff is None:
                        nc.scalar.copy(dst, pst[:])
                    else:
                        wsb, bsb = aff
                        nc.vector.tensor_scalar(dst, pst[:],
                                                wsb[:, hh:hh + 1],
                                                bsb[:, hh:hh + 1],
                                                ALU.mult, ALU.add)
                if mode == "qt":
                    nc.sync.dma_start(QT[:, :, st * 128:(st + 1) * 128],
                                      qstg[:])

            # matmuls with pipelined epilogue (one st behind)
            pending = None
            for st in range(n_st):
                ps = [psp.tile([128, W], FP, tag=f"c{i}", name=f"c{i}",
                               bufs=(2 if (NCH <= 2 or i < 2) else 1))
                      for i in range(NCH)]
                for ct in range(n_ct):
                    g, j = divmod(ct, gsz)
                    for i in range(NCH):
                        nc.tensor.matmul(
                            ps[i][:],
                            lhsT=xt_tiles[g][:, j, st * 128:(st + 1) * 128],
                            rhs=w_tiles[g][:, j, i * W:(i + 1) * W],
                            start=(ct == 0), stop=(ct == n_ct - 1),
                        )
                if pending is not None:
                    epilogue(*pending)
                pending = (ps, st)
            if pending is not None:
                epilogue(*pending)

        RG = [[2 * i, 2 * i + 1] for i in range(NCORES // 2)]
        NGRP = 4  # contraction-tile groups for pipelined loads

        def load_grouped(pool, dram, n_ct, dflat, tagp, engine=None):
            """Load [n_ct*128, dflat] weights as NGRP grouped tiles."""
            eng = engine or nc.gpsimd
            gsz = n_ct // NGRP
            tiles = []
            view = dram.rearrange("(g j p) d -> g p j d", p=128, j=gsz)
            for g in range(NGRP):
                t = pool.tile([128, gsz, dflat], BF, tag=f"{tagp}{g}",
                              name=f"{tagp}{g}")
                eng.dma_start(t[:], view[g])
                tiles.append(t)
            return tiles

        # ---------------- Phase A ----------------
        with ExitStack() as pa:
            xtp = pa.enter_context(tc.tile_pool(name="xtq", bufs=1))
            csp = pa.enter_context(tc.tile_pool(name="cs", bufs=1))
            cos_sb = csp.tile([128, NQ, D], FP, tag="cos", name="cos_sb")
            sin_sb = csp.tile([128, NQ, D], FP, tag="sin", name="sin_sb")
            nc.sync.dma_start(cos_sb[:],
                              cosq.rearrange("(st p) d -> p st d", p=128))
            nc.sync.dma_start(sin_sb[:],
                              sinq.rearrange("(st p) d -> p st d", p=128))
            xt_tiles = load_grouped(xtp, xTq, CT, SQ, "xt", engine=nc.sync)

            with ExitStack() as pkv:
                wkp = pkv.enter_context(tc.tile_pool(name="wkp", bufs=1))
                wk_tiles = load_grouped(wkp, wk, CT, KD, "wk")
                wv_tiles = load_grouped(wkp, wv, CT, KD, "wv")

                # K projection -> KT_sb own half + KT_loc -> AllGather
                def kt_dst(hh, st):
                    return KT_sb[:, hh, 0, st * 128:(st + 1) * 128]

                with ExitStack() as ph:
                    proj(ph, xt_tiles, CT, wk_tiles, KD, NQ, ln=True,
                         rope=True, cos_sb=cos_sb, sin_sb=sin_sb,
                         tgt=("kt", kt_dst), aff=affs.get("k"),
                         swap_eng=nc.vector)
                nc.sync.dma_start(KT_loc.rearrange("k d s -> d k s"),
                                  KT_sb[:, :, 0, :])
                nc.gpsimd.collective_compute(
                    "AllGather", ALU.bypass, replica_groups=RG,
                    ins=[KT_loc.opt()], outs=[KT_g.opt()])
                # both halves refreshed from gathered (SPMD-safe)
                for t in range(2):
                    nc.gpsimd.dma_start(
                        KT_sb[:, :, t, :],
                        KT_g[t].rearrange("k d s -> d k s"))

                # V projection -> V_res rows own half + V_loc -> AllGather
                def v_dst(st, i, Wl):
                    return V_res[:, st, i * Wl:(i + 1) * Wl]

                with ExitStack() as ph:
                    proj(ph, xt_tiles, CT, wv_tiles, KD, NQ, ln=False,
                         rope=False, cos_sb=None, sin_sb=None,
                         tgt=("vres", v_dst), aff=None)
                nc.sync.dma_start(
                    V_loc.rearrange("(st p) d -> p st d", p=128),
                    V_res[:, 0:NQ, :])
                nc.gpsimd.collective_compute(
                    "AllGather", ALU.bypass, replica_groups=RG,
                    ins=[V_loc.opt()], outs=[V_g.opt()])
                nc.gpsimd.dma_start(
                    V_res[:],
                    V_g.rearrange("t (st p) d -> p (t st) d", p=128))

            # Q projection (collectives overlap this; rope swaps on DVE so
            # nothing here queues behind the collectives on gpsimd)
            with ExitStack() as pq:
                wqp = pq.enter_context(tc.tile_pool(name="wqp", bufs=1))
                wq_tiles = load_grouped(wqp, wq, CT, HD, "wq",
                                        engine=nc.scalar)
                with ExitStack() as ph:
                    proj(ph, xt_tiles, CT, wq_tiles, HD, NQ, ln=True,
                         rope=True, cos_sb=cos_sb, sin_sb=sin_sb,
                         tgt=("qt", None), aff=affs.get("q"),
                         swap_eng=nc.vector)

        # caption projections (no collective; overlap AG tails)
        with ExitStack() as pc2:
            ctp = pc2.enter_context(tc.tile_pool(name="ct", bufs=1))
            cap_tiles = load_grouped(ctp, capT, CTC, LC_, "cap",
                                     engine=nc.scalar)
            wcp = pc2.enter_context(tc.tile_pool(name="wcp", bufs=1))
            wvc_tiles = load_grouped(wcp, wvc, CTC, KD, "wvc",
                                     engine=nc.scalar)
            wkc_tiles = load_grouped(wcp, wkc, CTC, KD, "wkc",
                                     engine=nc.scalar)

            def vc_dst(st, i, Wl):
                return Vc_res[:, st, i * Wl:(i + 1) * Wl]

            with ExitStack() as ph:
                proj(ph, cap_tiles, CTC, wvc_tiles, KD, NLC, ln=False,
                     rope=False, cos_sb=None, sin_sb=None,
                     tgt=("vres", vc_dst), aff=None)

            def kct_dst(hh, st):
                return KcT_sb[:, hh, st * 128:(st + 1) * 128]

            with ExitStack() as ph:
                proj(ph, cap_tiles, CTC, wkc_tiles, KD, NLC, ln=True,
                     rope=False, cos_sb=None, sin_sb=None,
                     tgt=("kt", kct_dst), aff=affs.get("kc"))

        # ---------------- Phase B: attention ----------------
        QCH = 512
        NQC = SQ // QCH
        NPAIR = NK // 2
        aTp = top.enter_context(tc.tile_pool(name="aTp", bufs=1))
        aT_sb = aTp.tile([128, H_, SQ], BF, tag="aT", name="aT_sb")
        wop = top.enter_context(tc.tile_pool(name="wop", bufs=1))
        wo_res = wop.tile([128, H_, HID_], BF, tag="wo", name="wo")
        wo_view = wo.rearrange("(g j p) e -> g p j e", p=128, j=H_ // NGRP)
        wo_tiles_view = wo_res.rearrange("p (g j) e -> g p j e",
                                         g=NGRP)
        for g in range(NGRP):
            nc.gpsimd.dma_start(wo_tiles_view[g], wo_view[g])

        with ExitStack() as pb:
            qp = pb.enter_context(tc.tile_pool(name="qw", bufs=2))
            ptp = pb.enter_context(tc.tile_pool(name="pt", bufs=3))
            pt8p = pb.enter_context(tc.tile_pool(name="pt8", bufs=3))
            scp = pb.enter_context(tc.tile_pool(name="sc2", bufs=2))
            ps_s = pb.enter_context(tc.tile_pool(name="ps_s", bufs=2,
                                                 space="PSUM"))
            ps_o = pb.enter_context(tc.tile_pool(name="ps_o", bufs=1,
                                                 space="PSUM"))
            ps_oc = pb.enter_context(tc.tile_pool(name="ps_oc", bufs=1,
                                                  space="PSUM"))
            ps_d = pb.enter_context(tc.tile_pool(name="ps_d", bufs=1,
                                                 space="PSUM"))
            ps_dc = pb.enter_context(tc.tile_pool(name="ps_dc", bufs=1,
                                                  space="PSUM"))

            for kv in range(KV_):
                for rep in range(H_ // KV_):
                    h = kv * (H_ // KV_) + rep
                    qtw = qp.tile([128, SQ], BF, tag="qtw", name="qtw")
                    nc.sync.dma_start(qtw[:], QT[:, h, :])
                    for ch in range(NQC):
                        qs = qtw[:, ch * QCH:(ch + 1) * QCH]
                        po = ps_o.tile([128, QCH], FP, tag="po", name="po")
                        poc = ps_oc.tile([128, QCH], FP, tag="poc",
                                         name="poc")
                        pden = ps_d.tile([128, QCH], FP, tag="pden",
                                         name="pden")
                        pdenc = ps_dc.tile([128, QCH], FP, tag="pdenc",
                                           name="pdenc")
                        pts = [None] * NPAIR
                        pt8s = [None] * NPAIR

                        def emit_scores(p):
                            psc = ps_s.tile([128, 2 * QCH], FP, tag="psc",
                                            name="psc")
                            for half in range(2):
                                kt = 2 * p + half
                                nc.tensor.matmul(
                                    psc[:, half * QCH:(half + 1) * QCH],
                                    lhsT=KT_sb[:, kv, kt // NQ,
                                               (kt % NQ) * 128:
                                               (kt % NQ + 1) * 128],
                                    rhs=qs, start=True, stop=True)
                            pt = ptp.tile([128, 2 * QCH], BF, tag="pt",
                                          name="pt")
                            if EXP_PAIR:
                                nc.scalar.activation(pt[:], psc[:], AF.Exp,
                                                     scale=SCALE)
                            else:
                                for half in range(2):
                                    sl = slice(half * QCH, (half + 1) * QCH)
                                    nc.scalar.activation(pt[:, sl],
                                                         psc[:, sl], AF.Exp,
                                                         scale=SCALE)
                            pts[p] = pt
                            if DEN_FP8:
                                pt8 = pt8p.tile([128, 2, QCH], F8,
                                                tag="pt8", name="pt8")
                                nc.vector.tensor_copy(
                                    pt8.rearrange("p two q -> p (two q)"),
                                    pt[:])
                                pt8s[p] = pt8

                        def emit_av(p):
                            pt = pts[p]
                            for half in range(2):
                                kt = 2 * p + half
                                nc.tensor.matmul(
                                    po[:],
                                    lhsT=V_res[:, kt,
                                               kv * 128:(kv + 1) * 128],
                                    rhs=pt[:, half * QCH:(half + 1) * QCH],
                                    start=(kt == 0), stop=(kt == NK - 1))

                        def emit_den(p):
                            if DEN_FP8:
                                pt8 = pt8s[p]
                                for qh in range(2):
                                    nc.tensor.matmul(
                                        pden[:, qh * 256:(qh + 1) * 256],
                                        lhsT=ones8[:],
                                        rhs=pt8[:, :, qh * 256:(qh + 1) * 256],
                                        start=(p == 0),
                                        stop=(p == NPAIR - 1),
                                        perf_mode=DR)
                            else:
                                pt = pts[p]
                                for half in range(2):
                                    kt = 2 * p + half
                                    nc.tensor.matmul(
                                        pden[:],
                                        lhsT=ones_bk[:],
                                        rhs=pt[:, half * QCH:(half + 1) * QCH],
                                        start=(kt == 0), stop=(kt == NK - 1))

                        for p in range(NPAIR):
                            emit_scores(p)
                            if p >= 1:
                                emit_av(p - 1)
                            if p >= 2:
                                emit_den(p - 2)
                        emit_av(NPAIR - 1)
                        emit_den(NPAIR - 2)
                        emit_den(NPAIR - 1)

                        # caption attention (one kt pair)
                        pscc = ps_s.tile([128, 2 * QCH], FP, tag="psc",
                                         name="pscc")
                        for half in range(NLC):
                            nc.tensor.matmul(
                                pscc[:, half * QCH:(half + 1) * QCH],
                                lhsT=KcT_sb[:, kv,
                                            half * 128:(half + 1) * 128],
                                rhs=qs, start=True, stop=True)
                        ptc = ptp.tile([128, 2 * QCH], BF, tag="pt",
                                       name="ptc")
                        if EXP_PAIR:
                            nc.scalar.activation(ptc[:], pscc[:], AF.Exp,
                                                 scale=SCALE)
                        else:
                            for half in range(2):
                                sl = slice(half * QCH, (half + 1) * QCH)
                                nc.scalar.activation(ptc[:, sl], pscc[:, sl],
                                                     AF.Exp, scale=SCALE)
                        if DEN_FP8:
                            pt8c = pt8p.tile([128, 2, QCH], F8, tag="pt8",
                                             name="pt8c")
                            nc.vector.tensor_copy(
                                pt8c.rearrange("p two q -> p (two q)"),
                                ptc[:])
                        for half in range(NLC):
                            nc.tensor.matmul(
                                poc[:],
                                lhsT=Vc_res[:, half,
                                            kv * 128:(kv + 1) * 128],
                                rhs=ptc[:, half * QCH:(half + 1) * QCH],
                                start=(half == 0), stop=(half == NLC - 1))
                        if DEN_FP8:
                            for qh in range(2):
                                nc.tensor.matmul(
                                    pdenc[:, qh * 256:(qh + 1) * 256],
                                    lhsT=ones8[:],
                                    rhs=pt8c[:, :, qh * 256:(qh + 1) * 256],
                                    start=True, stop=True, perf_mode=DR)
                        else:
                            for half in range(NLC):
                                nc.tensor.matmul(
                                    pdenc[:],
                                    lhsT=ones_bk[:],
                                    rhs=ptc[:, half * QCH:(half + 1) * QCH],
                                    start=(half == 0),
                                    stop=(half == NLC - 1))

                        # epilogue -> aT_sb
                        rden = scp.tile([128, QCH], FP, tag="rden",
                                        name="rden")
                        nc.vector.reciprocal_approx_fast(rden[:], pden[:])
                        t2 = scp.tile([128, QCH], FP, tag="t2", name="t2")
                        nc.vector.tensor_tensor(t2[:], po[:], rden[:],
                                                ALU.mult)
                        rdenc = scp.tile([128, QCH], FP, tag="rdenc",
                                         name="rdenc")
                        nc.vector.reciprocal_approx_fast(rdenc[:], pdenc[:])
                        tmp = scp.tile([128, QCH], FP, tag="tmp", name="tmp")
                        nc.vector.scalar_tensor_tensor(
                            tmp[:], poc[:], float(gate_t[h]), rdenc[:],
                            ALU.mult, ALU.mult)
                        nc.vector.tensor_tensor(
                            aT_sb[:, h, ch * QCH:(ch + 1) * QCH],
                            t2[:], tmp[:], ALU.add)

        # ---------------- Phase C: output projection ----------------
        with ExitStack() as pc:
            op_ = pc.enter_context(tc.tile_pool(name="osb", bufs=2))
            cps = pc.enter_context(tc.tile_pool(name="cps", bufs=2,
                                                space="PSUM"))
            EW = 512
            NEC = HID_ // EW
            for st in range(NQ):
                ostg = op_.tile([128, HID_], FP, tag="ostg", name="ostg")
                for ec in range(NEC):
                    pso = cps.tile([128, EW], FP, tag="cps", name="cps")
                    for hh in range(H_):
                        nc.tensor.matmul(
                            pso[:],
                            lhsT=aT_sb[:, hh, st * 128:(st + 1) * 128],
                            rhs=wo_res[:, hh, ec * EW:(ec + 1) * EW],
                            start=(hh == 0), stop=(hh == H_ - 1))
                    nc.scalar.copy(ostg[:, ec * EW:(ec + 1) * EW], pso[:])
                nc.sync.dma_start(out[st * 128:(st + 1) * 128, :], ostg[:])

    nc.compile()
    return nc


_CACHE = {}


def _get_program(cfg, gate_t, ln_trivial):
    key = (tuple(sorted(cfg.items())), tuple(np.round(gate_t, 8)), ln_trivial)
    if key not in _CACHE:
        _CACHE[key] = _build(cfg, gate_t, ln_trivial)
    return _CACHE[key]


def make_in_maps(cfg, inputs):
    """Host-side sharding: returns (in_maps, gate_t, ln_trivial)."""
    S_, SQ = cfg["S"], cfg["SQ"]
    x = np.asarray(inputs["x"], np.float32)
    cap = np.asarray(inputs["caption_feat"], np.float32)
    cos = np.ascontiguousarray(np.asarray(inputs["freqs_cos"], np.float32))
    sin = np.ascontiguousarray(np.asarray(inputs["freqs_sin"], np.float32))
    gate_t = np.tanh(np.asarray(inputs["gate"], np.float32))

    def bf(a):
        return np.ascontiguousarray(a).astype(BF16)

    def center(w):
        w = np.asarray(w, np.float32)
        return w - w.mean(axis=1, keepdims=True)

    weights = {
        "wq": bf(center(inputs["wq"])),
        "wk": bf(center(inputs["wk"])),
        "wv": bf(np.asarray(inputs["wv"], np.float32)),
        "wo": bf(np.asarray(inputs["wo"], np.float32)),
        "wkc": bf(center(inputs["wk_cap"])),
        "wvc": bf(np.asarray(inputs["wv_cap"], np.float32)),
    }

    lns = {}
    triv = []
    for nm, wk_, bk_ in (("q", "q_ln_w", "q_ln_b"), ("k", "k_ln_w", "k_ln_b"),
                         ("kc", "kc_ln_w", "kc_ln_b")):
        w = np.ascontiguousarray(np.asarray(inputs[wk_], np.float32))
        b = np.ascontiguousarray(np.asarray(inputs[bk_], np.float32))
        triv.append(bool(np.all(w == 1.0) and np.all(b == 0.0)))
        lns[f"ln_{nm}_w"] = w
        lns[f"ln_{nm}_b"] = b

    in_maps = []
    for c in range(NCORES):
        b_, half = divmod(c, 2)
        xTb = bf(x[b_].T)
        m = dict(
            xTq=np.ascontiguousarray(xTb[:, half * SQ:(half + 1) * SQ]),
            capT=bf(cap[b_].T),
            cosq=np.ascontiguousarray(
                np.repeat(cos[half * SQ:(half + 1) * SQ], 2, axis=1)),
            sinq=np.ascontiguousarray(
                np.repeat(sin[half * SQ:(half + 1) * SQ], 2, axis=1)
                * np.tile([-1.0, 1.0], cos.shape[1]).astype(np.float32)),
            **weights, **lns,
        )
        in_maps.append(m)
    return in_maps, gate_t, tuple(triv)


def _install_ntff_hook():
    """Shim the missing antenv.axon_hooks module so trace=True can capture
    NTFF profiles via the axon .so (test-time only)."""
    import types

    try:
        import antenv.axon_hooks  # noqa: F401
        return
    except ImportError:
        pass
    mod = types.ModuleType("antenv.axon_hooks")
    mod._hook = None

    def set_axon_ntff_profile_hook(h):
        mod._hook = h

    def get_axon_ntff_profile_hook():
        return mod._hook

    mod.set_axon_ntff_profile_hook = set_axon_ntff_profile_hook
    mod.get_axon_ntff_profile_hook = get_axon_ntff_profile_hook
    sys.modules["antenv.axon_hooks"] = mod
    import antenv
    antenv.axon_hooks = mod
    try:
        from trn_agent_boot.trn_boot import _ntff_profile_via_ctypes
        hook = _ntff_profile_via_ctypes("/opt/axon/libaxon_pjrt.so")
        if hook is not None:
            mod._hook = hook
    except Exception as e:  # degrade to no tracing
        print("ntff hook install failed:", e, file=sys.stderr)


def run_shards(cfg, inputs, trace=False):
    """Compile (cached), run on 8 cores, return (per-core outs, results)."""
    from concourse import bass_utils
    if trace:
        _install_ntff_hook()
    in_maps, gate_t, triv = make_in_maps(cfg, inputs)
    nc = _get_program(cfg, gate_t, triv)
    res = bass_utils.run_bass_kernel_spmd(
        nc, in_maps, core_ids=list(range(NCORES)), trace=trace)
    return [r["out"] for r in res.results], res


def kernel(**inputs):
    outs, _ = run_shards(FULL_CFG, inputs, trace=False)
    SQ = FULL_CFG["SQ"]
    full = np.empty((B, S, HID), np.float32)
    for c in range(NCORES):
        b_, half = divmod(c, 2)
        full[b_, half * SQ:(half + 1) * SQ, :] = outs[c]
    return full


# revision 35
# speedup vs baseline: 1.1125x; 1.0060x over previous
"""Trainium2 Bass kernel for nn_Attention_58360015618558 (v2).

Strategy (8 NeuronCores, SPMD):
  - Shard: core c -> (batch b = c//2, seq-half h = c%2); each core produces
    the output rows for its 1024 query positions.
  - K/V computed for the local seq-half only, AllGathered within the
    (even, odd) core pair; collectives overlap Q/caption projections.
  - LayerNorm mean folded into host-centered weights (z - mean(z) =
    x @ w_centered); LN rsqrt fused into the RoPE cos/sin multipliers, so
    the projection epilogue is 2 DVE passes + 2 gpsimd strided passes.
  - Attention in transposed-score layout (S^T[k,q]); exp on ACT over
    [128,1024] two-bank PSUM tiles; softmax denominator via fp8e5 (e5m2)
    DoubleRow matmuls (2x PE throughput; positive sums average the fp8
    quantization error down to ~0.2%).
  - K^T, V, Kc^T, Vc, and the attention output a^T all SBUF-resident;
    wo resident during phase B; single batched DMAs with multi-dim APs
    everywhere (DMA prepare costs ~1us of engine time per trigger).
"""

import math
import sys

import numpy as np

sys.path.insert(0, "/opt/trn_rl_repo")

import ml_dtypes  # noqa: E402

BF16 = ml_dtypes.bfloat16

# Full-size problem config
HID, H, KV, D, CAP = 2048, 16, 8, 128, 2048
B, S, LC = 4, 2048, 256
EPS = 1e-5
NCORES = 8

FULL_CFG = dict(S=S, SQ=S // 2, HID=HID, CAP=CAP, LC=LC, H=H, KV=KV)

# fp8e5 DoubleRow den: numerically wrong on HW (DoubleRow operand layout
# differs from the interp model) and no faster than bf16 -- keep off.
DEN_FP8 = False
EXP_PAIR = True   # single exp over [128,1024] two-bank PSUM tiles


def _build(cfg, gate_t, ln_trivial=(True, True, True)):
    """Build the per-core Bass program. Returns compiled Bacc."""
    import concourse.bass as bass  # noqa: F401
    import concourse.mybir as mybir
    import concourse.tile as tile
    from concourse import bacc
    from concourse.masks import make_identity
    from contextlib import ExitStack

    FP = mybir.dt.float32
    BF = mybir.dt.bfloat16
    F8 = mybir.dt.float8e5
    AF = mybir.ActivationFunctionType
    ALU = mybir.AluOpType
    DR = mybir.MatmulPerfMode.DoubleRow

    S_, SQ, HID_, CAP_, LC_ = cfg["S"], cfg["SQ"], cfg["HID"], cfg["CAP"], cfg["LC"]
    H_, KV_ = cfg["H"], cfg["KV"]
    HD, KD = H_ * D, KV_ * D
    CT, CTC = HID_ // 128, CAP_ // 128   # contraction tiles for x / caption
    NQ, NK, NLC = SQ // 128, S_ // 128, LC_ // 128
    SCALE = 1.0 / math.sqrt(D)
    qtriv, ktriv, kctriv = ln_trivial

    nc = bacc.Bacc("TRN2", target_bir_lowering=False, debug=False,
                   num_devices=NCORES)

    def din(name, shape, dt=BF):
        return nc.dram_tensor(name, shape, dt, kind="ExternalInput").ap()

    xTq = din("xTq", [HID_, SQ])        # x[b].T q-half columns (per core)
    capT = din("capT", [CAP_, LC_])
    wq = din("wq", [HID_, HD])          # column-centered host-side
    wk = din("wk", [HID_, KD])          # column-centered host-side
    wv = din("wv", [HID_, KD])
    wkc = din("wkc", [CAP_, KD])        # column-centered host-side
    wvc = din("wvc", [CAP_, KD])
    wo = din("wo", [HD, HID_])
    cosq = din("cosq", [SQ, D])   # interleaved: c[s,2i]=c[s,2i+1]=cos[s,i]
    sinq = din("sinq", [SQ, D])   # interleaved signed: -sin even, +sin odd
    lnw = {}
    for nm, dflat in (("q", HD), ("k", KD), ("kc", KD)):
        lnw[nm] = (din(f"ln_{nm}_w", [dflat], FP), din(f"ln_{nm}_b", [dflat], FP))
    out = nc.dram_tensor("out", [SQ, HID_], FP, kind="ExternalOutput").ap()

    # SPMD: all 8 cores run one graph.  Local K/V halves are staged to
    # _loc DRAM, AllGathered within the core pair, and both halves are
    # re-filled from the gathered tensor (rank-independent).  Collective
    # completion is enforced by gpsimd queue order (fills on gpsimd).
    with ExitStack() as top:
        tc = top.enter_context(tile.TileContext(nc))

        constp = top.enter_context(tc.tile_pool(name="const", bufs=1))
        resp = top.enter_context(tc.tile_pool(name="res", bufs=1))
        dramp = top.enter_context(tc.tile_pool(name="dram", bufs=1, space="DRAM"))

        ident = constp.tile([128, 128], BF, tag="ident", name="ident")
        make_identity(nc, ident[:])
        ones8 = constp.tile([128, 2, 128], F8, tag="ones8", name="ones8")
        nc.vector.memset(ones8[:], 1.0)
        ones_bk = constp.tile([128, 128], BF, tag="ones_bk", name="ones_bk")
        nc.vector.memset(ones_bk[:], 1.0)
        zero_c = constp.tile([128, 1], FP, tag="zero_c", name="zero_c")
        nc.vector.memset(zero_c[:], 0.0)
        nc.const_aps.aps[(FP, 0.0)] = zero_c[:]
        for ci, v in enumerate({float(KD * EPS), float(HD * EPS)}):
            ec_ = constp.tile([128, 1], FP, tag=f"eps{ci}", name=f"eps{ci}")
            nc.vector.memset(ec_[:], v)
            nc.const_aps.aps[(FP, v)] = ec_[:]

        # LN affine params (only loaded when nontrivial)
        affs = {}
        for nm, dflat, triv in (("q", HD, qtriv), ("k", KD, ktriv),
                                ("kc", KD, kctriv)):
            if not triv:
                wsb = constp.tile([128, dflat // 128], FP, tag=f"aw_{nm}",
                                  name=f"aw_{nm}")
                bsb = constp.tile([128, dflat // 128], FP, tag=f"ab_{nm}",
                                  name=f"ab_{nm}")
                nc.gpsimd.dma_start(wsb[:], lnw[nm][0].rearrange(
                    "(o p) -> p o", p=128))
                nc.gpsimd.dma_start(bsb[:], lnw[nm][1].rearrange(
                    "(o p) -> p o", p=128))
                affs[nm] = (wsb, bsb)

        # SBUF residents (V_res's pool opens later, after wq frees)
        KT_sb = resp.tile([128, KV_, 2, SQ], BF, tag="KT_sb", name="KT_sb")
        KcT_sb = resp.tile([128, KV_, LC_], BF, tag="KcT_sb", name="KcT_sb")
        Vc_res = resp.tile([128, NLC, KD], BF, tag="Vc_res", name="Vc_res")

        # DRAM intermediates
        KT_loc = dramp.tile([KV_, 128, SQ], BF, tag="KT_loc", name="KT_loc")
        KT_g = dramp.tile([2, KV_, 128, SQ], BF, tag="KT_g", name="KT_g")
        V_loc = dramp.tile([SQ, KD], BF, tag="V_loc", name="V_loc")
        V_g = dramp.tile([2, SQ, KD], BF, tag="V_g", name="V_g")
        QT = dramp.tile([128, H_, SQ], BF, tag="QT", name="QT")  # [d, h, s]

        # ---------------- projection pass ----------------
        def proj(ctx, xt_tiles, n_ct, w_tiles, dflat, n_st, ln, rope,
                 cos_sb, sin_sb, tgt, aff, swap_eng=None, lag=1,
                 ps_bufs=None):
            """One projection with fused LN(+RoPE) epilogue.

            xt_tiles/w_tiles: list of SBUF tiles [128, g, *] covering n_ct
              contraction tiles (group size g).
            tgt: ("kt", kt_dst_fn)   hh -> AP to write [128,128] transposed
                 ("qt", None)        -> stage [128,H,128] + 1 DMA to QT
                 ("vres", slot_ap_fn) st, i, W -> AP for plain copy
            lag: epilogue runs `lag` st-iterations behind the matmuls
              (needs ps_bufs >= lag+1 to avoid PE stalls).
            """
            W = 512
            NCH = dflat // W
            gszx = n_ct // len(xt_tiles)
            gszw = n_ct // len(w_tiles)
            if ps_bufs is None:
                ps_bufs = [2] * NCH
            sbufs = lag + 1
            psp = ctx.enter_context(tc.tile_pool(name="pjps", bufs=1,
                                                 space="PSUM"))
            tpp = ctx.enter_context(tc.tile_pool(name="tp", bufs=2,
                                                 space="PSUM"))
            sc = ctx.enter_context(tc.tile_pool(name="pjsc", bufs=2))
            qsp = ctx.enter_context(tc.tile_pool(name="qstg", bufs=2))

            SQRTD = math.sqrt(float(dflat))

            def epilogue(ps, st):
                nh = dflat // 128
                if ln:
                    # variance of the (already mean-centered) projection
                    stats = sc.tile([128, 8], FP, tag="stats", name="stats")
                    sqj = sc.tile([128, W], FP, tag="sqj", name="sqj")
                    for i in range(NCH):
                        nc.scalar.activation(sqj[:], ps[i][:], AF.Square,
                                             accum_out=stats[:, i:i + 1])
                    width = 1
                    while width < NCH:
                        for i in range(0, NCH, 2 * width):
                            if i + width < NCH:
                                nc.vector.tensor_tensor(
                                    stats[:, i:i + 1], stats[:, i:i + 1],
                                    stats[:, i + width:i + width + 1], ALU.add)
                        width *= 2
                    nc.scalar.activation(stats[:, 6:7], stats[:, 0:1],
                                         AF.Sqrt, bias=float(dflat * EPS))
                    rs = stats[:, 7:8]
                    nc.vector.reciprocal(rs, stats[:, 6:7])

                roped = sc.tile([128, dflat], BF, tag="roped", name="roped",
                                bufs=sbufs)
                if rope:
                    # o = z*(rs*c*sqrt(d)) + swap(z)*(rs*s_signed*sqrt(d))
                    rc = sc.tile([128, D], FP, tag="rc", name="rc")
                    rsig = sc.tile([128, D], FP, tag="rsig", name="rsig")
                    nc.vector.tensor_scalar(rc[:], cos_sb[:, st, :], rs,
                                            SQRTD, ALU.mult, ALU.mult)
                    nc.vector.tensor_scalar(rsig[:], sin_sb[:, st, :], rs,
                                            SQRTD, ALU.mult, ALU.mult)
                    rce = rc[:, None, :]
                    sw = sc.tile([128, dflat], BF, tag="sw", name="sw",
                                 bufs=sbufs)
                    rsig_v = rsig.rearrange("p (i two) -> p i two", two=2)
                    for i in range(NCH):
                        wh = W // D
                        rv = roped[:, i * W:(i + 1) * W].rearrange(
                            "p (h d) -> p h d", h=wh)
                        pv = ps[i].rearrange("p (h d) -> p h d", h=wh)
                        nc.vector.tensor_tensor(
                            rv, pv, rce.to_broadcast([128, wh, D]), ALU.mult)
                        p2 = ps[i].rearrange("p (h i two) -> p h i two",
                                             two=2, h=wh)
                        s2 = sw[:, i * W:(i + 1) * W].rearrange(
                            "p (h i two) -> p h i two", two=2, h=wh)
                        swap_eng.tensor_tensor(
                            s2[:, :, :, 0], p2[:, :, :, 1],
                            rsig_v[:, None, :, 0].to_broadcast(
                                [128, wh, D // 2]), ALU.mult)
                        swap_eng.tensor_tensor(
                            s2[:, :, :, 1], p2[:, :, :, 0],
                            rsig_v[:, None, :, 1].to_broadcast(
                                [128, wh, D // 2]), ALU.mult)
                        nc.vector.tensor_tensor(
                            roped[:, i * W:(i + 1) * W],
                            roped[:, i * W:(i + 1) * W],
                            sw[:, i * W:(i + 1) * W], ALU.add)
                elif ln:
                    for i in range(NCH):
                        nc.vector.tensor_scalar(
                            roped[:, i * W:(i + 1) * W], ps[i][:], rs,
                            SQRTD, ALU.mult, ALU.mult)
                else:
                    mode, dst_fn = tgt
                    if mode == "vdram":
                        vstg = qsp.tile([128, dflat], BF, tag="vstg",
                                        name="vstg")
                        for i in range(NCH):
                            nc.scalar.copy(vstg[:, i * W:(i + 1) * W],
                                           ps[i][:])
                        nc.sync.dma_start(
                            dst_fn[st * 128:(st + 1) * 128, :], vstg[:])
                    else:
                        for i in range(NCH):
                            nc.scalar.copy(dst_fn(st, i, W), ps[i][:])
                    return

                # transpose + write out
                mode, dst_fn = tgt
                qstg = None
                if mode == "qt":
                    qstg = qsp.tile([128, nh, 128], BF, tag="qstg",
                                    name="qstg")
                for hh in range(nh):
                    pst = tpp.tile([128, 128], BF, tag="tp", name="tp")
                    nc.tensor.transpose(pst[:],
                                        roped[:, hh * 128:(hh + 1) * 128],
                                        ident[:])
                    if mode == "qt":
                        dst = qstg[:, hh, :]
                    else:
                        dst = dst_fn(hh, st)
                    if aff is None:
                        nc.scalar.copy(dst, pst[:])
                    else:
                        wsb, bsb = aff
                        nc.vector.tensor_scalar(dst, pst[:],
                                                wsb[:, hh:hh + 1],
                                                bsb[:, hh:hh + 1],
                                                ALU.mult, ALU.add)
                if mode == "qt":
                    nc.sync.dma_start(QT[:, :, st * 128:(st + 1) * 128],
                                      qstg[:])

            # matmuls with pipelined epilogue (`lag` sts behind)
            pending = []
            for st in range(n_st):
                ps = [psp.tile([128, W], FP, tag=f"c{i}", name=f"c{i}",
                               bufs=ps_bufs[i])
                      for i in range(NCH)]
                for ct in range(n_ct):
                    gx, jx = divmod(ct, gszx)
                    gw, jw = divmod(ct, gszw)
                    for i in range(NCH):
                        nc.tensor.matmul(
                            ps[i][:],
                            lhsT=xt_tiles[gx][:, jx, st * 128:(st + 1) * 128],
                            rhs=w_tiles[gw][:, jw, i * W:(i + 1) * W],
                            start=(ct == 0), stop=(ct == n_ct - 1),
                        )
                pending.append((ps, st))
                if len(pending) > lag:
                    epilogue(*pending.pop(0))
            for args in pending:
                epilogue(*args)

        RG = [[2 * i, 2 * i + 1] for i in range(NCORES // 2)]
        NGRP = 4  # contraction-tile groups for pipelined loads

        def load_grouped(pool, dram, n_ct, dflat, tagp, engine=None,
                         ngrp=NGRP):
            """Load [n_ct*128, dflat] weights as ngrp grouped tiles."""
            eng = engine or nc.gpsimd
            gsz = n_ct // ngrp
            tiles = []
            view = dram.rearrange("(g j p) d -> g p j d", p=128, j=gsz)
            for g in range(ngrp):
                t = pool.tile([128, gsz, dflat], BF, tag=f"{tagp}{g}",
                              name=f"{tagp}{g}")
                eng.dma_start(t[:], view[g])
                tiles.append(t)
            return tiles

        # ---------------- Phase A ----------------
        # Order: K (collective earliest) -> Q (big, hides AG-K) -> V ->
        # caption.  wq loads issue at t=0 on the idle vector queue; xtq/wk
        # split 8 ways for a fast start.  V_res's pool opens after wq frees.
        with ExitStack() as pa:
            xtp = pa.enter_context(tc.tile_pool(name="xtq", bufs=1))
            csp = pa.enter_context(tc.tile_pool(name="cs", bufs=1))
            cos_sb = csp.tile([128, NQ, D], BF, tag="cos", name="cos_sb")
            sin_sb = csp.tile([128, NQ, D], BF, tag="sin", name="sin_sb")
            nc.sync.dma_start(cos_sb[:],
                              cosq.rearrange("(st p) d -> p st d", p=128))
            nc.sync.dma_start(sin_sb[:],
                              sinq.rearrange("(st p) d -> p st d", p=128))
            xt_tiles = load_grouped(xtp, xTq, CT, SQ, "xt", engine=nc.sync,
                                    ngrp=8)
            wq_stack = ExitStack()
            wqp = wq_stack.enter_context(tc.tile_pool(name="wqp", bufs=1))
            wq_tiles = load_grouped(wqp, wq, CT, HD, "wq", engine=nc.scalar)

            with ExitStack() as pk:
                wkp = pk.enter_context(tc.tile_pool(name="wkp", bufs=1))
                wk_tiles = load_grouped(wkp, wk, CT, KD, "wk", ngrp=8)

                # K projection -> KT_sb own half + KT_loc -> AllGather
                def kt_dst(hh, st):
                    return KT_sb[:, hh, 0, st * 128:(st + 1) * 128]

                with ExitStack() as ph:
                    proj(ph, xt_tiles, CT, wk_tiles, KD, NQ, ln=True,
                         rope=True, cos_sb=cos_sb, sin_sb=sin_sb,
                         tgt=("kt", kt_dst), aff=affs.get("k"),
                         swap_eng=nc.vector, lag=2, ps_bufs=[3, 3])
                nc.sync.dma_start(KT_loc.rearrange("k d s -> d k s"),
                                  KT_sb[:, :, 0, :])
                nc.gpsimd.collective_compute(
                    "AllGather", ALU.bypass, replica_groups=RG,
                    ins=[KT_loc.opt()], outs=[KT_g.opt()])
                # both halves refreshed from gathered (SPMD-safe)
                for t in range(2):
                    nc.gpsimd.dma_start(
                        KT_sb[:, :, t, :],
                        KT_g[t].rearrange("k d s -> d k s"))

            # Q projection (AG-K overlaps this; rope swaps on DVE so
            # nothing here queues behind the collectives on gpsimd)
            with ExitStack() as ph:
                proj(ph, xt_tiles, CT, wq_tiles, HD, NQ, ln=True,
                     rope=True, cos_sb=cos_sb, sin_sb=sin_sb,
                     tgt=("qt", None), aff=affs.get("q"),
                     swap_eng=nc.vector, lag=1, ps_bufs=[2, 2, 1, 1])
            wq_stack.close()

            with ExitStack() as pt2:
                # caption loads issued early (scalar queue) so the caption
                # projections don't stall after V
                ctp = pt2.enter_context(tc.tile_pool(name="ct", bufs=1))
                wvcp = pt2.enter_context(tc.tile_pool(name="wvcp", bufs=1))
                cap_tiles = load_grouped(ctp, capT, CTC, LC_, "cap",
                                         engine=nc.scalar)
                wvc_tiles = load_grouped(wvcp, wvc, CTC, KD, "wvc",
                                         engine=nc.scalar)

                # V projection -> V_loc rows (DRAM) -> AllGather
                with ExitStack() as pv:
                    wvp = pv.enter_context(tc.tile_pool(name="wvp", bufs=1))
                    wv_tiles = load_grouped(wvp, wv, CT, KD, "wv")
                    with ExitStack() as ph:
                        proj(ph, xt_tiles, CT, wv_tiles, KD, NQ, ln=False,
                             rope=False, cos_sb=None, sin_sb=None,
                             tgt=("vdram", V_loc), aff=None)
                    nc.gpsimd.collective_compute(
                        "AllGather", ALU.bypass, replica_groups=RG,
                        ins=[V_loc.opt()], outs=[V_g.opt()])

                # caption projections (overlap the AG-V tail)
                wkcp = pt2.enter_context(tc.tile_pool(name="wkcp", bufs=1))
                wkc_tiles = load_grouped(wkcp, wkc, CTC, KD, "wkc",
                                         engine=nc.scalar)

                def vc_dst(st, i, Wl):
                    return Vc_res[:, st, i * Wl:(i + 1) * Wl]

                with ExitStack() as ph:
                    proj(ph, cap_tiles, CTC, wvc_tiles, KD, NLC, ln=False,
                         rope=False, cos_sb=None, sin_sb=None,
                         tgt=("vres", vc_dst), aff=None)

                def kct_dst(hh, st):
                    return KcT_sb[:, hh, st * 128:(st + 1) * 128]

                with ExitStack() as ph:
                    proj(ph, cap_tiles, CTC, wkc_tiles, KD, NLC, ln=True,
                         rope=False, cos_sb=None, sin_sb=None,
                         tgt=("kt", kct_dst), aff=affs.get("kc"))

        # ---------------- Phase B: attention ----------------
        # V_res opens only now (phase-A pools closed) and fills from V_g
        vresp = top.enter_context(tc.tile_pool(name="vres", bufs=1))
        V_res = vresp.tile([128, NK, KD], BF, tag="V_res", name="V_res")
        nc.gpsimd.dma_start(
            V_res[:], V_g.rearrange("t (st p) d -> p (t st) d", p=128))

        QCH = 512
        NQC = SQ // QCH
        NPAIR = NK // 2
        aTp = top.enter_context(tc.tile_pool(name="aTp", bufs=1))
        aT_sb = aTp.tile([128, H_, SQ], BF, tag="aT", name="aT_sb")
        wop = top.enter_context(tc.tile_pool(name="wop", bufs=1))
        wo_res = wop.tile([128, H_, HID_], BF, tag="wo", name="wo")
        wo_view = wo.rearrange("(g j p) e -> g p j e", p=128, j=H_ // NGRP)
        wo_tiles_view = wo_res.rearrange("p (g j) e -> g p j e",
                                         g=NGRP)
        for g in range(NGRP):
            nc.gpsimd.dma_start(wo_tiles_view[g], wo_view[g])

        with ExitStack() as pb:
            qp = pb.enter_context(tc.tile_pool(name="qw", bufs=2))
            ptp = pb.enter_context(tc.tile_pool(name="pt", bufs=3))
            pt8p = pb.enter_context(tc.tile_pool(name="pt8", bufs=3))
            scp = pb.enter_context(tc.tile_pool(name="sc2", bufs=2))
            ps_s = pb.enter_context(tc.tile_pool(name="ps_s", bufs=2,
                                                 space="PSUM"))
            ps_o = pb.enter_context(tc.tile_pool(name="ps_o", bufs=1,
                                                 space="PSUM"))
            ps_oc = pb.enter_context(tc.tile_pool(name="ps_oc", bufs=1,
                                                  space="PSUM"))
            ps_d = pb.enter_context(tc.tile_pool(name="ps_d", bufs=1,
                                                 space="PSUM"))
            ps_dc = pb.enter_context(tc.tile_pool(name="ps_dc", bufs=1,
                                                  space="PSUM"))

            for kv in range(KV_):
                for rep in range(H_ // KV_):
                    h = kv * (H_ // KV_) + rep
                    qtw = qp.tile([128, SQ], BF, tag="qtw", name="qtw")
                    nc.sync.dma_start(qtw[:], QT[:, h, :])
                    for ch in range(NQC):
                        qs = qtw[:, ch * QCH:(ch + 1) * QCH]
                        po = ps_o.tile([128, QCH], FP, tag="po", name="po")
                        poc = ps_oc.tile([128, QCH], FP, tag="poc",
                                         name="poc")
                        pden = ps_d.tile([128, QCH], FP, tag="pden",
                                         name="pden")
                        pdenc = ps_dc.tile([128, QCH], FP, tag="pdenc",
                                           name="pdenc")
                        pts = [None] * NPAIR
                        pt8s = [None] * NPAIR

                        def emit_scores(p):
                            psc = ps_s.tile([128, 2 * QCH], FP, tag="psc",
                                            name="psc")
                            for half in range(2):
                                kt = 2 * p + half
                                nc.tensor.matmul(
                                    psc[:, half * QCH:(half + 1) * QCH],
                                    lhsT=KT_sb[:, kv, kt // NQ,
                                               (kt % NQ) * 128:
                                               (kt % NQ + 1) * 128],
                                    rhs=qs, start=True, stop=True)
                            pt = ptp.tile([128, 2 * QCH], BF, tag="pt",
                                          name="pt")
                            if EXP_PAIR:
                                nc.scalar.activation(pt[:], psc[:], AF.Exp,
                                                     scale=SCALE)
                            else:
                                for half in range(2):
                                    sl = slice(half * QCH, (half + 1) * QCH)
                                    nc.scalar.activation(pt[:, sl],
                                                         psc[:, sl], AF.Exp,
                                                         scale=SCALE)
                            pts[p] = pt
                            if DEN_FP8:
                                pt8 = pt8p.tile([128, 2, QCH], F8,
                                                tag="pt8", name="pt8")
                                nc.vector.tensor_copy(
                                    pt8.rearrange("p two q -> p (two q)"),
                                    pt[:])
                                pt8s[p] = pt8

                        def emit_av(p):
                            pt = pts[p]
                            for half in range(2):
                                kt = 2 * p + half
                                nc.tensor.matmul(
                                    po[:],
                                    lhsT=V_res[:, kt,
                                               kv * 128:(kv + 1) * 128],
                                    rhs=pt[:, half * QCH:(half + 1) * QCH],
                                    start=(kt == 0), stop=(kt == NK - 1))

                        def emit_den(p):
                            if DEN_FP8:
                                pt8 = pt8s[p]
                                for qh in range(2):
                                    nc.tensor.matmul(
                                        pden[:, qh * 256:(qh + 1) * 256],
                                        lhsT=ones8[:],
                                        rhs=pt8[:, :, qh * 256:(qh + 1) * 256],
                                        start=(p == 0),
                                        stop=(p == NPAIR - 1),
                                        perf_mode=DR)
                            else:
                                pt = pts[p]
                                for half in range(2):
                                    kt = 2 * p + half
                                    nc.tensor.matmul(
                                        pden[:],
                                        lhsT=ones_bk[:],
                                        rhs=pt[:, half * QCH:(half + 1) * QCH],
                                        start=(kt == 0), stop=(kt == NK - 1))

                        for p in range(NPAIR):
                            emit_scores(p)
                            if p >= 1:
                                emit_av(p - 1)
                            if p >= 2:
                                emit_den(p - 2)
                        emit_av(NPAIR - 1)
                        emit_den(NPAIR - 2)
                        emit_den(NPAIR - 1)

                        # caption attention (one kt pair)
                        pscc = ps_s.tile([128, 2 * QCH], FP, tag="psc",
                                         name="pscc")
                        for half in range(NLC):
                            nc.tensor.matmul(
                                pscc[:, half * QCH:(half + 1) * QCH],
                                lhsT=KcT_sb[:, kv,
                                            half * 128:(half + 1) * 128],
                                rhs=qs, start=True, stop=True)
                        ptc = ptp.tile([128, 2 * QCH], BF, tag="pt",
                                       name="ptc")
                        if EXP_PAIR:
                            nc.scalar.activation(ptc[:], pscc[:], AF.Exp,
                                                 scale=SCALE)
                        else:
                            for half in range(2):
                                sl = slice(half * QCH, (half + 1) * QCH)
                                nc.scalar.activation(ptc[:, sl], pscc[:, sl],
                                                     AF.Exp, scale=SCALE)
                        if DEN_FP8:
                            pt8c = pt8p.tile([128, 2, QCH], F8, tag="pt8",
                                             name="pt8c")
                            nc.vector.tensor_copy(
                                pt8c.rearrange("p two q -> p (two q)"),
                                ptc[:])
                        for half in range(NLC):
                            nc.tensor.matmul(
                                poc[:],
                                lhsT=Vc_res[:, half,
                                            kv * 128:(kv + 1) * 128],
                                rhs=ptc[:, half * QCH:(half + 1) * QCH],
                                start=(half == 0), stop=(half == NLC - 1))
                        if DEN_FP8:
                            for qh in range(2):
                                nc.tensor.matmul(
                                    pdenc[:, qh * 256:(qh + 1) * 256],
                                    lhsT=ones8[:],
                                    rhs=pt8c[:, :, qh * 256:(qh + 1) * 256],
                                    start=True, stop=True, perf_mode=DR)
                        else:
                            for half in range(NLC):
                                nc.tensor.matmul(
                                    pdenc[:],
                                    lhsT=ones_bk[:],
                                    rhs=ptc[:, half * QCH:(half + 1) * QCH],
                                    start=(half == 0),
                                    stop=(half == NLC - 1))

                        # epilogue -> aT_sb
                        rden = scp.tile([128, QCH], FP, tag="rden",
                                        name="rden")
                        nc.vector.reciprocal_approx_fast(rden[:], pden[:])
                        t2 = scp.tile([128, QCH], FP, tag="t2", name="t2")
                        nc.vector.tensor_tensor(t2[:], po[:], rden[:],
                                                ALU.mult)
                        rdenc = scp.tile([128, QCH], FP, tag="rdenc",
                                         name="rdenc")
                        nc.vector.reciprocal_approx_fast(rdenc[:], pdenc[:])
                        tmp = scp.tile([128, QCH], FP, tag="tmp", name="tmp")
                        nc.vector.scalar_tensor_tensor(
                            tmp[:], poc[:], float(gate_t[h]), rdenc[:],
                            ALU.mult, ALU.mult)
                        nc.vector.tensor_tensor(
                            aT_sb[:, h, ch * QCH:(ch + 1) * QCH],
                            t2[:], tmp[:], ALU.add)

        # ---------------- Phase C: output projection ----------------
        with ExitStack() as pc:
            op_ = pc.enter_context(tc.tile_pool(name="osb", bufs=2))
            cps = pc.enter_context(tc.tile_pool(name="cps", bufs=2,
                                                space="PSUM"))
            EW = 512
            NEC = HID_ // EW
            for st in range(NQ):
                ostg = op_.tile([128, HID_], FP, tag="ostg", name="ostg")
                for ec in range(NEC):
                    pso = cps.tile([128, EW], FP, tag="cps", name="cps")
                    for hh in range(H_):
                        nc.tensor.matmul(
                            pso[:],
                            lhsT=aT_sb[:, hh, st * 128:(st + 1) * 128],
                            rhs=wo_res[:, hh, ec * EW:(ec + 1) * EW],
                            start=(hh == 0), stop=(hh == H_ - 1))
                    nc.scalar.copy(ostg[:, ec * EW:(ec + 1) * EW], pso[:])
                nc.sync.dma_start(out[st * 128:(st + 1) * 128, :], ostg[:])

    nc.compile()
    return nc


_CACHE = {}


def _get_program(cfg, gate_t, ln_trivial):
    key = (tuple(sorted(cfg.items())), tuple(np.round(gate_t, 8)), ln_trivial)
    if key not in _CACHE:
        _CACHE[key] = _build(cfg, gate_t, ln_trivial)
    return _CACHE[key]


def make_in_maps(cfg, inputs):
    """Host-side sharding: returns (in_maps, gate_t, ln_trivial)."""
    S_, SQ = cfg["S"], cfg["SQ"]
    x = np.asarray(inputs["x"], np.float32)
    cap = np.asarray(inputs["caption_feat"], np.float32)
    cos = np.ascontiguousarray(np.asarray(inputs["freqs_cos"], np.float32))
    sin = np.ascontiguousarray(np.asarray(inputs["freqs_sin"], np.float32))
    gate_t = np.tanh(np.asarray(inputs["gate"], np.float32))

    def bf(a):
        return np.ascontiguousarray(a).astype(BF16)

    def center(w):
        w = np.asarray(w, np.float32)
        return w - w.mean(axis=1, keepdims=True)

    weights = {
        "wq": bf(center(inputs["wq"])),
        "wk": bf(center(inputs["wk"])),
        "wv": bf(np.asarray(inputs["wv"], np.float32)),
        "wo": bf(np.asarray(inputs["wo"], np.float32)),
        "wkc": bf(center(inputs["wk_cap"])),
        "wvc": bf(np.asarray(inputs["wv_cap"], np.float32)),
    }

    lns = {}
    triv = []
    for nm, wk_, bk_ in (("q", "q_ln_w", "q_ln_b"), ("k", "k_ln_w", "k_ln_b"),
                         ("kc", "kc_ln_w", "kc_ln_b")):
        w = np.ascontiguousarray(np.asarray(inputs[wk_], np.float32))
        b = np.ascontiguousarray(np.asarray(inputs[bk_], np.float32))
        triv.append(bool(np.all(w == 1.0) and np.all(b == 0.0)))
        lns[f"ln_{nm}_w"] = w
        lns[f"ln_{nm}_b"] = b

    in_maps = []
    for c in range(NCORES):
        b_, half = divmod(c, 2)
        xTb = bf(x[b_].T)
        m = dict(
            xTq=np.ascontiguousarray(xTb[:, half * SQ:(half + 1) * SQ]),
            capT=bf(cap[b_].T),
            cosq=bf(np.repeat(cos[half * SQ:(half + 1) * SQ], 2, axis=1)),
            sinq=bf(np.repeat(sin[half * SQ:(half + 1) * SQ], 2, axis=1)
                    * np.tile([-1.0, 1.0], cos.shape[1]).astype(np.float32)),
            **weights, **lns,
        )
        in_maps.append(m)
    return in_maps, gate_t, tuple(triv)


def _install_ntff_hook():
    """Shim the missing antenv.axon_hooks module so trace=True can capture
    NTFF profiles via the axon .so (test-time only)."""
    import types

    try:
        import antenv.axon_hooks  # noqa: F401
        return
    except ImportError:
        pass
    mod = types.ModuleType("antenv.axon_hooks")
    mod._hook = None

    def set_axon_ntff_profile_hook(h):
        mod._hook = h

    def get_axon_ntff_profile_hook():
        return mod._hook

    mod.set_axon_ntff_profile_hook = set_axon_ntff_profile_hook
    mod.get_axon_ntff_profile_hook = get_axon_ntff_profile_hook
    sys.modules["antenv.axon_hooks"] = mod
    import antenv
    antenv.axon_hooks = mod
    try:
        from trn_agent_boot.trn_boot import _ntff_profile_via_ctypes
        hook = _ntff_profile_via_ctypes("/opt/axon/libaxon_pjrt.so")
        if hook is not None:
            mod._hook = hook
    except Exception as e:  # degrade to no tracing
        print("ntff hook install failed:", e, file=sys.stderr)


def run_shards(cfg, inputs, trace=False):
    """Compile (cached), run on 8 cores, return (per-core outs, results)."""
    from concourse import bass_utils
    if trace:
        _install_ntff_hook()
    in_maps, gate_t, triv = make_in_maps(cfg, inputs)
    nc = _get_program(cfg, gate_t, triv)
    res = bass_utils.run_bass_kernel_spmd(
        nc, in_maps, core_ids=list(range(NCORES)), trace=trace)
    return [r["out"] for r in res.results], res


def kernel(**inputs):
    outs, _ = run_shards(FULL_CFG, inputs, trace=False)
    SQ = FULL_CFG["SQ"]
    full = np.empty((B, S, HID), np.float32)
    for c in range(NCORES):
        b_, half = divmod(c, 2)
        full[b_, half * SQ:(half + 1) * SQ, :] = outs[c]
    return full


# revision 46
# speedup vs baseline: 1.1421x; 1.0266x over previous
"""Trainium2 Bass kernel for nn_Attention_58360015618558 (v2).

Strategy (8 NeuronCores, SPMD):
  - Shard: core c -> (batch b = c//2, seq-half h = c%2); each core produces
    the output rows for its 1024 query positions.
  - K/V computed for the local seq-half only, AllGathered within the
    (even, odd) core pair; collectives overlap Q/caption projections.
  - LayerNorm mean folded into host-centered weights (z - mean(z) =
    x @ w_centered); LN rsqrt fused into the RoPE cos/sin multipliers, so
    the projection epilogue is 2 DVE passes + 2 gpsimd strided passes.
  - Attention in transposed-score layout (S^T[k,q]); exp on ACT over
    [128,1024] two-bank PSUM tiles; softmax denominator via fp8e5 (e5m2)
    DoubleRow matmuls (2x PE throughput; positive sums average the fp8
    quantization error down to ~0.2%).
  - K^T, V, Kc^T, Vc, and the attention output a^T all SBUF-resident;
    wo resident during phase B; single batched DMAs with multi-dim APs
    everywhere (DMA prepare costs ~1us of engine time per trigger).
"""

import math
import sys

import numpy as np

sys.path.insert(0, "/opt/trn_rl_repo")

import ml_dtypes  # noqa: E402

BF16 = ml_dtypes.bfloat16

# Full-size problem config
HID, H, KV, D, CAP = 2048, 16, 8, 128, 2048
B, S, LC = 4, 2048, 256
EPS = 1e-5
NCORES = 8

FULL_CFG = dict(S=S, SQ=S // 2, HID=HID, CAP=CAP, LC=LC, H=H, KV=KV)

# fp8e5 DoubleRow den: numerically wrong on HW (DoubleRow operand layout
# differs from the interp model) and no faster than bf16 -- keep off.
DEN_FP8 = False
EXP_PAIR = True   # single exp over [128,1024] two-bank PSUM tiles


def _build(cfg, gate_t, ln_trivial=(True, True, True)):
    """Build the per-core Bass program. Returns compiled Bacc."""
    import concourse.bass as bass  # noqa: F401
    import concourse.mybir as mybir
    import concourse.tile as tile
    from concourse import bacc
    from concourse.masks import make_identity
    from contextlib import ExitStack

    FP = mybir.dt.float32
    BF = mybir.dt.bfloat16
    F8 = mybir.dt.float8e5
    AF = mybir.ActivationFunctionType
    ALU = mybir.AluOpType
    DR = mybir.MatmulPerfMode.DoubleRow

    S_, SQ, HID_, CAP_, LC_ = cfg["S"], cfg["SQ"], cfg["HID"], cfg["CAP"], cfg["LC"]
    H_, KV_ = cfg["H"], cfg["KV"]
    HD, KD = H_ * D, KV_ * D
    CT, CTC = HID_ // 128, CAP_ // 128   # contraction tiles for x / caption
    NQ, NK, NLC = SQ // 128, S_ // 128, LC_ // 128
    SCALE = 1.0 / math.sqrt(D)
    qtriv, ktriv, kctriv = ln_trivial

    nc = bacc.Bacc("TRN2", target_bir_lowering=False, debug=False,
                   num_devices=NCORES)

    def din(name, shape, dt=BF):
        return nc.dram_tensor(name, shape, dt, kind="ExternalInput").ap()

    xTq = din("xTq", [HID_, SQ])        # x[b].T q-half columns (per core)
    capT = din("capT", [CAP_, LC_])
    wq = din("wq", [HID_, HD])          # column-centered host-side
    wk = din("wk", [HID_, KD])          # column-centered host-side
    wv = din("wv", [HID_, KD])
    wkc = din("wkc", [CAP_, KD])        # column-centered host-side
    wvc = din("wvc", [CAP_, KD])
    wo = din("wo", [HD, HID_])
    cosq = din("cosq", [SQ, D])   # interleaved: c[s,2i]=c[s,2i+1]=cos[s,i]
    sinq = din("sinq", [SQ, D])   # interleaved signed: -sin even, +sin odd
    lnw = {}
    for nm, dflat in (("q", HD), ("k", KD), ("kc", KD)):
        lnw[nm] = (din(f"ln_{nm}_w", [dflat], FP), din(f"ln_{nm}_b", [dflat], FP))
    out = nc.dram_tensor("out", [SQ, HID_], FP, kind="ExternalOutput").ap()

    # SPMD: all 8 cores run one graph.  Local K/V halves are staged to
    # _loc DRAM, AllGathered within the core pair, and both halves are
    # re-filled from the gathered tensor (rank-independent).  Collective
    # completion is enforced by gpsimd queue order (fills on gpsimd).
    with ExitStack() as top:
        tc = top.enter_context(tile.TileContext(nc))

        constp = top.enter_context(tc.tile_pool(name="const", bufs=1))
        resp = top.enter_context(tc.tile_pool(name="res", bufs=1))
        dramp = top.enter_context(tc.tile_pool(name="dram", bufs=1, space="DRAM"))

        ident = constp.tile([128, 128], BF, tag="ident", name="ident")
        make_identity(nc, ident[:])
        ones8 = constp.tile([128, 2, 128], F8, tag="ones8", name="ones8")
        nc.vector.memset(ones8[:], 1.0)
        ones_bk = constp.tile([128, 128], BF, tag="ones_bk", name="ones_bk")
        nc.vector.memset(ones_bk[:], 1.0)
        zero_c = constp.tile([128, 1], FP, tag="zero_c", name="zero_c")
        nc.vector.memset(zero_c[:], 0.0)
        nc.const_aps.aps[(FP, 0.0)] = zero_c[:]
        for ci, v in enumerate({float(KD * EPS), float(HD * EPS)}):
            ec_ = constp.tile([128, 1], FP, tag=f"eps{ci}", name=f"eps{ci}")
            nc.vector.memset(ec_[:], v)
            nc.const_aps.aps[(FP, v)] = ec_[:]

        # LN affine params (only loaded when nontrivial)
        affs = {}
        for nm, dflat, triv in (("q", HD, qtriv), ("k", KD, ktriv),
                                ("kc", KD, kctriv)):
            if not triv:
                wsb = constp.tile([128, dflat // 128], FP, tag=f"aw_{nm}",
                                  name=f"aw_{nm}")
                bsb = constp.tile([128, dflat // 128], FP, tag=f"ab_{nm}",
                                  name=f"ab_{nm}")
                nc.gpsimd.dma_start(wsb[:], lnw[nm][0].rearrange(
                    "(o p) -> p o", p=128))
                nc.gpsimd.dma_start(bsb[:], lnw[nm][1].rearrange(
                    "(o p) -> p o", p=128))
                affs[nm] = (wsb, bsb)

        # SBUF residents (V_res's pool opens later, after wq frees)
        KT_sb = resp.tile([128, KV_, 2, SQ], BF, tag="KT_sb", name="KT_sb")
        KcT_sb = resp.tile([128, KV_, LC_], BF, tag="KcT_sb", name="KcT_sb")
        Vc_res = resp.tile([128, NLC, KD], BF, tag="Vc_res", name="Vc_res")

        # DRAM intermediates
        KT_loc = dramp.tile([KV_, 128, SQ], BF, tag="KT_loc", name="KT_loc")
        KT_g = dramp.tile([2, KV_, 128, SQ], BF, tag="KT_g", name="KT_g")
        V_loc = dramp.tile([SQ, KD], BF, tag="V_loc", name="V_loc")
        V_g = dramp.tile([2, SQ, KD], BF, tag="V_g", name="V_g")
        QT = dramp.tile([128, H_, SQ], BF, tag="QT", name="QT")  # [d, h, s]

        # ---------------- projection pass ----------------
        def proj(ctx, xt_tiles, n_ct, w_tiles, dflat, n_st, ln, rope,
                 cos_sb, sin_sb, tgt, aff, swap_eng=None, lag=1,
                 ps_bufs=None):
            """One projection with fused LN(+RoPE) epilogue.

            xt_tiles/w_tiles: list of SBUF tiles [128, g, *] covering n_ct
              contraction tiles (group size g).
            tgt: ("kt", kt_dst_fn)   hh -> AP to write [128,128] transposed
                 ("qt", None)        -> stage [128,H,128] + 1 DMA to QT
                 ("vres", slot_ap_fn) st, i, W -> AP for plain copy
            lag: epilogue runs `lag` st-iterations behind the matmuls
              (needs ps_bufs >= lag+1 to avoid PE stalls).
            """
            W = 512
            NCH = dflat // W
            gszx = n_ct // len(xt_tiles)
            gszw = n_ct // len(w_tiles)
            if ps_bufs is None:
                ps_bufs = [2] * NCH
            sbufs = lag + 1
            psp = ctx.enter_context(tc.tile_pool(name="pjps", bufs=1,
                                                 space="PSUM"))
            tpp = ctx.enter_context(tc.tile_pool(name="tp", bufs=2,
                                                 space="PSUM"))
            sc = ctx.enter_context(tc.tile_pool(name="pjsc", bufs=2))
            qsp = ctx.enter_context(tc.tile_pool(name="qstg", bufs=2))

            SQRTD = math.sqrt(float(dflat))

            def epilogue(ps, st):
                nh = dflat // 128
                if ln:
                    # variance of the (already mean-centered) projection
                    stats = sc.tile([128, 8], FP, tag="stats", name="stats")
                    sqj = sc.tile([128, W], FP, tag="sqj", name="sqj")
                    for i in range(NCH):
                        nc.scalar.activation(sqj[:], ps[i][:], AF.Square,
                                             accum_out=stats[:, i:i + 1])
                    width = 1
                    while width < NCH:
                        for i in range(0, NCH, 2 * width):
                            if i + width < NCH:
                                nc.vector.tensor_tensor(
                                    stats[:, i:i + 1], stats[:, i:i + 1],
                                    stats[:, i + width:i + width + 1], ALU.add)
                        width *= 2
                    nc.scalar.activation(stats[:, 6:7], stats[:, 0:1],
                                         AF.Sqrt, bias=float(dflat * EPS))
                    rs = stats[:, 7:8]
                    nc.vector.reciprocal(rs, stats[:, 6:7])

                roped = sc.tile([128, dflat], BF, tag="roped", name="roped",
                                bufs=sbufs)
                if rope:
                    # o = z*(rs*c*sqrt(d)) + swap(z)*(rs*s_signed*sqrt(d))
                    rc = sc.tile([128, D], FP, tag="rc", name="rc")
                    rsig = sc.tile([128, D], FP, tag="rsig", name="rsig")
                    nc.vector.tensor_scalar(rc[:], cos_sb[:, st, :], rs,
                                            SQRTD, ALU.mult, ALU.mult)
                    nc.vector.tensor_scalar(rsig[:], sin_sb[:, st, :], rs,
                                            SQRTD, ALU.mult, ALU.mult)
                    rce = rc[:, None, :]
                    sw = sc.tile([128, dflat], BF, tag="sw", name="sw",
                                 bufs=sbufs)
                    rsig_v = rsig.rearrange("p (i two) -> p i two", two=2)
                    for i in range(NCH):
                        wh = W // D
                        rv = roped[:, i * W:(i + 1) * W].rearrange(
                            "p (h d) -> p h d", h=wh)
                        pv = ps[i].rearrange("p (h d) -> p h d", h=wh)
                        nc.vector.tensor_tensor(
                            rv, pv, rce.to_broadcast([128, wh, D]), ALU.mult)
                        p2 = ps[i].rearrange("p (h i two) -> p h i two",
                                             two=2, h=wh)
                        s2 = sw[:, i * W:(i + 1) * W].rearrange(
                            "p (h i two) -> p h i two", two=2, h=wh)
                        swap_eng.tensor_tensor(
                            s2[:, :, :, 0], p2[:, :, :, 1],
                            rsig_v[:, None, :, 0].to_broadcast(
                                [128, wh, D // 2]), ALU.mult)
                        swap_eng.tensor_tensor(
                            s2[:, :, :, 1], p2[:, :, :, 0],
                            rsig_v[:, None, :, 1].to_broadcast(
                                [128, wh, D // 2]), ALU.mult)
                        nc.vector.tensor_tensor(
                            roped[:, i * W:(i + 1) * W],
                            roped[:, i * W:(i + 1) * W],
                            sw[:, i * W:(i + 1) * W], ALU.add)
                elif ln:
                    for i in range(NCH):
                        nc.vector.tensor_scalar(
                            roped[:, i * W:(i + 1) * W], ps[i][:], rs,
                            SQRTD, ALU.mult, ALU.mult)
                else:
                    mode, dst_fn = tgt
                    if mode == "vdram":
                        vstg = qsp.tile([128, dflat], BF, tag="vstg",
                                        name="vstg")
                        for i in range(NCH):
                            nc.scalar.copy(vstg[:, i * W:(i + 1) * W],
                                           ps[i][:])
                        nc.sync.dma_start(
                            dst_fn[st * 128:(st + 1) * 128, :], vstg[:])
                    else:
                        for i in range(NCH):
                            nc.scalar.copy(dst_fn(st, i, W), ps[i][:])
                    return

                # transpose + write out
                mode, dst_fn = tgt
                qstg = None
                if mode == "qt":
                    qstg = qsp.tile([128, nh, 128], BF, tag="qstg",
                                    name="qstg")
                for hh in range(nh):
                    pst = tpp.tile([128, 128], BF, tag="tp", name="tp")
                    nc.tensor.transpose(pst[:],
                                        roped[:, hh * 128:(hh + 1) * 128],
                                        ident[:])
                    if mode == "qt":
                        dst = qstg[:, hh, :]
                    else:
                        dst = dst_fn(hh, st)
                    if aff is None:
                        nc.scalar.copy(dst, pst[:])
                    else:
                        wsb, bsb = aff
                        nc.vector.tensor_scalar(dst, pst[:],
                                                wsb[:, hh:hh + 1],
                                                bsb[:, hh:hh + 1],
                                                ALU.mult, ALU.add)
                if mode == "qt":
                    nc.sync.dma_start(QT[:, :, st * 128:(st + 1) * 128],
                                      qstg[:])

            # matmuls with pipelined epilogue (`lag` sts behind)
            pending = []
            for st in range(n_st):
                ps = [psp.tile([128, W], FP, tag=f"c{i}", name=f"c{i}",
                               bufs=ps_bufs[i])
                      for i in range(NCH)]
                for ct in range(n_ct):
                    gx, jx = divmod(ct, gszx)
                    gw, jw = divmod(ct, gszw)
                    for i in range(NCH):
                        nc.tensor.matmul(
                            ps[i][:],
                            lhsT=xt_tiles[gx][:, jx, st * 128:(st + 1) * 128],
                            rhs=w_tiles[gw][:, jw, i * W:(i + 1) * W],
                            start=(ct == 0), stop=(ct == n_ct - 1),
                        )
                pending.append((ps, st))
                if len(pending) > lag:
                    epilogue(*pending.pop(0))
            for args in pending:
                epilogue(*args)

        RG = [[2 * i, 2 * i + 1] for i in range(NCORES // 2)]
        NGRP = 4  # contraction-tile groups for pipelined loads

        # collectives signal completion on semaphores so the (fast) HWDGE
        # queues can issue the SBUF fills; the gpsimd SWDGE prepare costs
        # ~18ns/descriptor (~37us for a 2048-descriptor fill).
        agk_sem = top.enter_context(nc.semaphore(name="agk_sem"))
        agv_sem = top.enter_context(nc.semaphore(name="agv_sem"))
        nc.scalar.sem_clear(agk_sem)
        nc.sync.sem_clear(agv_sem)

        def load_grouped(pool, dram, n_ct, dflat, tagp, engine=None,
                         ngrp=NGRP):
            """Load [n_ct*128, dflat] weights as ngrp grouped tiles."""
            eng = engine or nc.gpsimd
            gsz = n_ct // ngrp
            tiles = []
            view = dram.rearrange("(g j p) d -> g p j d", p=128, j=gsz)
            for g in range(ngrp):
                t = pool.tile([128, gsz, dflat], BF, tag=f"{tagp}{g}",
                              name=f"{tagp}{g}")
                eng.dma_start(t[:], view[g])
                tiles.append(t)
            return tiles

        # ---------------- Phase A ----------------
        # Order: K (collective earliest) -> Q (big, hides AG-K) -> V ->
        # caption.  wq loads issue at t=0 on the idle vector queue; xtq/wk
        # split 8 ways for a fast start.  V_res's pool opens after wq frees.
        with ExitStack() as pa:
            xtp = pa.enter_context(tc.tile_pool(name="xtq", bufs=1))
            csp = pa.enter_context(tc.tile_pool(name="cs", bufs=1))
            cos_sb = csp.tile([128, NQ, D], BF, tag="cos", name="cos_sb")
            sin_sb = csp.tile([128, NQ, D], BF, tag="sin", name="sin_sb")
            nc.sync.dma_start(cos_sb[:],
                              cosq.rearrange("(st p) d -> p st d", p=128))
            nc.sync.dma_start(sin_sb[:],
                              sinq.rearrange("(st p) d -> p st d", p=128))
            # xtq: 8 row-groups x 2 column-halves on two HWDGE queues, so
            # the first K matmuls (needing only st0's columns) start early
            gszx = CT // 8
            xview = xTq.rearrange("(g j p) s -> g p j s", p=128, j=gszx)
            xt_tiles = []
            for g in range(8):
                t = xtp.tile([128, gszx, SQ], BF, tag=f"xt{g}",
                             name=f"xt{g}")
                nc.sync.dma_start(t[:, :, 0:SQ // 2],
                                  xview[g][:, :, 0:SQ // 2])
                nc.scalar.dma_start(t[:, :, SQ // 2:SQ],
                                    xview[g][:, :, SQ // 2:SQ])
                xt_tiles.append(t)
            wq_stack = ExitStack()
            wqp = wq_stack.enter_context(tc.tile_pool(name="wqp", bufs=1))
            wq_tiles = load_grouped(wqp, wq, CT, HD, "wq", engine=nc.scalar)

            with ExitStack() as pk:
                wkp = pk.enter_context(tc.tile_pool(name="wkp", bufs=1))
                wk_tiles = load_grouped(wkp, wk, CT, KD, "wk", ngrp=8)

                # K projection -> KT_sb own half + KT_loc -> AllGather
                def kt_dst(hh, st):
                    return KT_sb[:, hh, 0, st * 128:(st + 1) * 128]

                with ExitStack() as ph:
                    proj(ph, xt_tiles, CT, wk_tiles, KD, NQ, ln=True,
                         rope=True, cos_sb=cos_sb, sin_sb=sin_sb,
                         tgt=("kt", kt_dst), aff=affs.get("k"),
                         swap_eng=nc.vector, lag=2, ps_bufs=[3, 3])
                nc.sync.dma_start(KT_loc.rearrange("k d s -> d k s"),
                                  KT_sb[:, :, 0, :])
                nc.gpsimd.collective_compute(
                    "AllGather", ALU.bypass, replica_groups=RG,
                    ins=[KT_loc.opt()], outs=[KT_g.opt()])
                # gpsimd queue order == collective completion; signal it
                nc.gpsimd.sem_inc(agk_sem, 1)
                # both halves refreshed from gathered (SPMD-safe); HWDGE
                # fills gated on the collective via agk_sem
                nc.scalar.wait_ge(agk_sem, 1)
                for t in range(2):
                    nc.scalar.dma_start(
                        KT_sb[:, :, t, :],
                        KT_g[t].rearrange("k d s -> d k s"))

            # Q projection (AG-K overlaps this; rope swaps on DVE so
            # nothing here queues behind the collectives on gpsimd)
            with ExitStack() as ph:
                proj(ph, xt_tiles, CT, wq_tiles, HD, NQ, ln=True,
                     rope=True, cos_sb=cos_sb, sin_sb=sin_sb,
                     tgt=("qt", None), aff=affs.get("q"),
                     swap_eng=nc.vector, lag=1, ps_bufs=[2, 2, 1, 1])
            wq_stack.close()

            with ExitStack() as pt2:
                # caption loads issued early (scalar queue) so the caption
                # projections don't stall after V
                ctp = pt2.enter_context(tc.tile_pool(name="ct", bufs=1))
                wvcp = pt2.enter_context(tc.tile_pool(name="wvcp", bufs=1))
                cap_tiles = load_grouped(ctp, capT, CTC, LC_, "cap",
                                         engine=nc.scalar)
                wvc_tiles = load_grouped(wvcp, wvc, CTC, KD, "wvc",
                                         engine=nc.scalar)

                # V projection -> V_loc rows (DRAM) -> AllGather
                with ExitStack() as pv:
                    wvp = pv.enter_context(tc.tile_pool(name="wvp", bufs=1))
                    wv_tiles = load_grouped(wvp, wv, CT, KD, "wv")
                    with ExitStack() as ph:
                        proj(ph, xt_tiles, CT, wv_tiles, KD, NQ, ln=False,
                             rope=False, cos_sb=None, sin_sb=None,
                             tgt=("vdram", V_loc), aff=None)
                    nc.gpsimd.collective_compute(
                        "AllGather", ALU.bypass, replica_groups=RG,
                        ins=[V_loc.opt()], outs=[V_g.opt()])
                    nc.gpsimd.sem_inc(agv_sem, 1)

                # caption projections (overlap the AG-V tail)
                wkcp = pt2.enter_context(tc.tile_pool(name="wkcp", bufs=1))
                wkc_tiles = load_grouped(wkcp, wkc, CTC, KD, "wkc",
                                         engine=nc.scalar)

                def vc_dst(st, i, Wl):
                    return Vc_res[:, st, i * Wl:(i + 1) * Wl]

                with ExitStack() as ph:
                    proj(ph, cap_tiles, CTC, wvc_tiles, KD, NLC, ln=False,
                         rope=False, cos_sb=None, sin_sb=None,
                         tgt=("vres", vc_dst), aff=None)

                def kct_dst(hh, st):
                    return KcT_sb[:, hh, st * 128:(st + 1) * 128]

                with ExitStack() as ph:
                    proj(ph, cap_tiles, CTC, wkc_tiles, KD, NLC, ln=True,
                         rope=False, cos_sb=None, sin_sb=None,
                         tgt=("kt", kct_dst), aff=affs.get("kc"),
                         lag=2, ps_bufs=[3, 3])

        # ---------------- Phase B: attention ----------------
        # V_res opens only now (phase-A pools closed) and fills from V_g.
        # Per-kv column fills on the sync HWDGE queue (gated on the AG-V
        # semaphore) so kv=0 is ready first, right as phase B needs it.
        vresp = top.enter_context(tc.tile_pool(name="vres", bufs=1))
        V_res = vresp.tile([128, NK, KD], BF, tag="V_res", name="V_res")
        V_g_r = V_g.rearrange("t (st p) d -> p (t st) d", p=128)
        nc.sync.wait_ge(agv_sem, 1)
        for kvf in range(KV_):
            nc.sync.dma_start(
                V_res[:, :, kvf * 128:(kvf + 1) * 128],
                V_g_r[:, :, kvf * 128:(kvf + 1) * 128])

        QCH = 512
        NQC = SQ // QCH
        NPAIR = NK // 2
        aTp = top.enter_context(tc.tile_pool(name="aTp", bufs=1))
        aT_sb = aTp.tile([128, H_, SQ], BF, tag="aT", name="aT_sb")
        wop = top.enter_context(tc.tile_pool(name="wop", bufs=1))
        wo_res = wop.tile([128, H_, HID_], BF, tag="wo", name="wo")
        wo_view = wo.rearrange("(g j p) e -> g p j e", p=128, j=H_ // NGRP)
        wo_tiles_view = wo_res.rearrange("p (g j) e -> g p j e",
                                         g=NGRP)
        for g in range(NGRP):
            nc.gpsimd.dma_start(wo_tiles_view[g], wo_view[g])

        with ExitStack() as pb:
            qp = pb.enter_context(tc.tile_pool(name="qw", bufs=2))
            ptp = pb.enter_context(tc.tile_pool(name="pt", bufs=NPAIR + 1))
            pt8p = pb.enter_context(tc.tile_pool(name="pt8", bufs=3))
            scp = pb.enter_context(tc.tile_pool(name="sc2", bufs=2))
            ps_s = pb.enter_context(tc.tile_pool(name="ps_s", bufs=2,
                                                 space="PSUM"))
            ps_o = pb.enter_context(tc.tile_pool(name="ps_o", bufs=1,
                                                 space="PSUM"))
            ps_oc = pb.enter_context(tc.tile_pool(name="ps_oc", bufs=1,
                                                  space="PSUM"))
            ps_d = pb.enter_context(tc.tile_pool(name="ps_d", bufs=1,
                                                 space="PSUM"))
            ps_dc = pb.enter_context(tc.tile_pool(name="ps_dc", bufs=1,
                                                  space="PSUM"))

            for kv in range(KV_):
                for rep in range(H_ // KV_):
                    h = kv * (H_ // KV_) + rep
                    qtw = qp.tile([128, SQ], BF, tag="qtw", name="qtw")
                    nc.sync.dma_start(qtw[:], QT[:, h, :])
                    for ch in range(NQC):
                        qs = qtw[:, ch * QCH:(ch + 1) * QCH]
                        po = ps_o.tile([128, QCH], FP, tag="po", name="po")
                        poc = ps_oc.tile([128, QCH], FP, tag="poc",
                                         name="poc")
                        pden = ps_d.tile([128, QCH], FP, tag="pden",
                                         name="pden")
                        pdenc = ps_dc.tile([128, QCH], FP, tag="pdenc",
                                           name="pdenc")
                        pts = [None] * NPAIR
                        pt8s = [None] * NPAIR

                        def emit_scores(p):
                            psc = ps_s.tile([128, 2 * QCH], FP, tag="psc",
                                            name="psc")
                            for half in range(2):
                                kt = 2 * p + half
                                nc.tensor.matmul(
                                    psc[:, half * QCH:(half + 1) * QCH],
                                    lhsT=KT_sb[:, kv, kt // NQ,
                                               (kt % NQ) * 128:
                                               (kt % NQ + 1) * 128],
                                    rhs=qs, start=True, stop=True)
                            pt = ptp.tile([128, 2 * QCH], BF, tag="pt",
                                          name="pt")
                            if EXP_PAIR:
                                nc.scalar.activation(pt[:], psc[:], AF.Exp,
                                                     scale=SCALE)
                            else:
                                for half in range(2):
                                    sl = slice(half * QCH, (half + 1) * QCH)
                                    nc.scalar.activation(pt[:, sl],
                                                         psc[:, sl], AF.Exp,
                                                         scale=SCALE)
                            pts[p] = pt
                            if DEN_FP8:
                                pt8 = pt8p.tile([128, 2, QCH], F8,
                                                tag="pt8", name="pt8")
                                nc.vector.tensor_copy(
                                    pt8.rearrange("p two q -> p (two q)"),
                                    pt[:])
                                pt8s[p] = pt8

                        def emit_av(p):
                            pt = pts[p]
                            for half in range(2):
                                kt = 2 * p + half
                                nc.tensor.matmul(
                                    po[:],
                                    lhsT=V_res[:, kt,
                                               kv * 128:(kv + 1) * 128],
                                    rhs=pt[:, half * QCH:(half + 1) * QCH],
                                    start=(kt == 0), stop=(kt == NK - 1))

                        def emit_den(p):
                            if DEN_FP8:
                                pt8 = pt8s[p]
                                for qh in range(2):
                                    nc.tensor.matmul(
                                        pden[:, qh * 256:(qh + 1) * 256],
                                        lhsT=ones8[:],
                                        rhs=pt8[:, :, qh * 256:(qh + 1) * 256],
                                        start=(p == 0),
                                        stop=(p == NPAIR - 1),
                                        perf_mode=DR)
                            else:
                                pt = pts[p]
                                for half in range(2):
                                    kt = 2 * p + half
                                    nc.tensor.matmul(
                                        pden[:],
                                        lhsT=ones_bk[:],
                                        rhs=pt[:, half * QCH:(half + 1) * QCH],
                                        start=(kt == 0), stop=(kt == NK - 1))

                        # all scores first: the first AV then comes ~8us
                        # into each block, absorbing V_res fill latency at
                        # phase-B start and keeping PE fed
                        for p in range(NPAIR):
                            emit_scores(p)
                        for p in range(NPAIR):
                            emit_av(p)
                            emit_den(p)

                        # caption attention (one kt pair)
                        pscc = ps_s.tile([128, 2 * QCH], FP, tag="psc",
                                         name="pscc")
                        for half in range(NLC):
                            nc.tensor.matmul(
                                pscc[:, half * QCH:(half + 1) * QCH],
                                lhsT=KcT_sb[:, kv,
                                            half * 128:(half + 1) * 128],
                                rhs=qs, start=True, stop=True)
                        ptc = ptp.tile([128, 2 * QCH], BF, tag="pt",
                                       name="ptc")
                        if EXP_PAIR:
                            nc.scalar.activation(ptc[:], pscc[:], AF.Exp,
                                                 scale=SCALE)
                        else:
                            for half in range(2):
                                sl = slice(half * QCH, (half + 1) * QCH)
                                nc.scalar.activation(ptc[:, sl], pscc[:, sl],
                                                     AF.Exp, scale=SCALE)
                        if DEN_FP8:
                            pt8c = pt8p.tile([128, 2, QCH], F8, tag="pt8",
                                             name="pt8c")
                            nc.vector.tensor_copy(
                                pt8c.rearrange("p two q -> p (two q)"),
                                ptc[:])
                        for half in range(NLC):
                            nc.tensor.matmul(
                                poc[:],
                                lhsT=Vc_res[:, half,
                                            kv * 128:(kv + 1) * 128],
                                rhs=ptc[:, half * QCH:(half + 1) * QCH],
                                start=(half == 0), stop=(half == NLC - 1))
                        if DEN_FP8:
                            for qh in range(2):
                                nc.tensor.matmul(
                                    pdenc[:, qh * 256:(qh + 1) * 256],
                                    lhsT=ones8[:],
                                    rhs=pt8c[:, :, qh * 256:(qh + 1) * 256],
                                    start=True, stop=True, perf_mode=DR)
                        else:
                            for half in range(NLC):
                                nc.tensor.matmul(
                                    pdenc[:],
                                    lhsT=ones_bk[:],
                                    rhs=ptc[:, half * QCH:(half + 1) * QCH],
                                    start=(half == 0),
                                    stop=(half == NLC - 1))

                        # epilogue -> aT_sb
                        aslice = aT_sb[:, h, ch * QCH:(ch + 1) * QCH]
                        rden = scp.tile([128, QCH], FP, tag="rden",
                                        name="rden")
                        nc.vector.reciprocal_approx_fast(rden[:], pden[:])
                        rdenc = scp.tile([128, QCH], FP, tag="rdenc",
                                         name="rdenc")
                        nc.vector.reciprocal_approx_fast(rdenc[:], pdenc[:])
                        tmp = scp.tile([128, QCH], FP, tag="tmp", name="tmp")
                        nc.vector.scalar_tensor_tensor(
                            tmp[:], poc[:], float(gate_t[h]), rdenc[:],
                            ALU.mult, ALU.mult)
                        nc.vector.tensor_tensor(aslice, po[:], rden[:],
                                                ALU.mult)
                        nc.vector.tensor_tensor(aslice, aslice, tmp[:],
                                                ALU.add)

        # ---------------- Phase C: output projection ----------------
        with ExitStack() as pc:
            op_ = pc.enter_context(tc.tile_pool(name="osb", bufs=2))
            cps = pc.enter_context(tc.tile_pool(name="cps", bufs=2,
                                                space="PSUM"))
            EW = 512
            NEC = HID_ // EW
            for st in range(NQ):
                ostg = op_.tile([128, HID_], FP, tag="ostg", name="ostg")
                for ec in range(NEC):
                    pso = cps.tile([128, EW], FP, tag="cps", name="cps")
                    for hh in range(H_):
                        nc.tensor.matmul(
                            pso[:],
                            lhsT=aT_sb[:, hh, st * 128:(st + 1) * 128],
                            rhs=wo_res[:, hh, ec * EW:(ec + 1) * EW],
                            start=(hh == 0), stop=(hh == H_ - 1))
                    nc.scalar.copy(ostg[:, ec * EW:(ec + 1) * EW], pso[:])
                nc.sync.dma_start(out[st * 128:(st + 1) * 128, :], ostg[:])

    nc.compile()
    return nc


_CACHE = {}


def _get_program(cfg, gate_t, ln_trivial):
    key = (tuple(sorted(cfg.items())), tuple(np.round(gate_t, 8)), ln_trivial)
    if key not in _CACHE:
        _CACHE[key] = _build(cfg, gate_t, ln_trivial)
    return _CACHE[key]


def make_in_maps(cfg, inputs):
    """Host-side sharding: returns (in_maps, gate_t, ln_trivial)."""
    S_, SQ = cfg["S"], cfg["SQ"]
    x = np.asarray(inputs["x"], np.float32)
    cap = np.asarray(inputs["caption_feat"], np.float32)
    cos = np.ascontiguousarray(np.asarray(inputs["freqs_cos"], np.float32))
    sin = np.ascontiguousarray(np.asarray(inputs["freqs_sin"], np.float32))
    gate_t = np.tanh(np.asarray(inputs["gate"], np.float32))

    def bf(a):
        return np.ascontiguousarray(a).astype(BF16)

    def center(w):
        w = np.asarray(w, np.float32)
        return w - w.mean(axis=1, keepdims=True)

    weights = {
        "wq": bf(center(inputs["wq"])),
        "wk": bf(center(inputs["wk"])),
        "wv": bf(np.asarray(inputs["wv"], np.float32)),
        "wo": bf(np.asarray(inputs["wo"], np.float32)),
        "wkc": bf(center(inputs["wk_cap"])),
        "wvc": bf(np.asarray(inputs["wv_cap"], np.float32)),
    }

    lns = {}
    triv = []
    for nm, wk_, bk_ in (("q", "q_ln_w", "q_ln_b"), ("k", "k_ln_w", "k_ln_b"),
                         ("kc", "kc_ln_w", "kc_ln_b")):
        w = np.ascontiguousarray(np.asarray(inputs[wk_], np.float32))
        b = np.ascontiguousarray(np.asarray(inputs[bk_], np.float32))
        triv.append(bool(np.all(w == 1.0) and np.all(b == 0.0)))
        lns[f"ln_{nm}_w"] = w
        lns[f"ln_{nm}_b"] = b

    in_maps = []
    for c in range(NCORES):
        b_, half = divmod(c, 2)
        xTb = bf(x[b_].T)
        m = dict(
            xTq=np.ascontiguousarray(xTb[:, half * SQ:(half + 1) * SQ]),
            capT=bf(cap[b_].T),
            cosq=bf(np.repeat(cos[half * SQ:(half + 1) * SQ], 2, axis=1)),
            sinq=bf(np.repeat(sin[half * SQ:(half + 1) * SQ], 2, axis=1)
                    * np.tile([-1.0, 1.0], cos.shape[1]).astype(np.float32)),
            **weights, **lns,
        )
        in_maps.append(m)
    return in_maps, gate_t, tuple(triv)


def _install_ntff_hook():
    """Shim the missing antenv.axon_hooks module so trace=True can capture
    NTFF profiles via the axon .so (test-time only)."""
    import types

    try:
        import antenv.axon_hooks  # noqa: F401
        return
    except ImportError:
        pass
    mod = types.ModuleType("antenv.axon_hooks")
    mod._hook = None

    def set_axon_ntff_profile_hook(h):
        mod._hook = h

    def get_axon_ntff_profile_hook():
        return mod._hook

    mod.set_axon_ntff_profile_hook = set_axon_ntff_profile_hook
    mod.get_axon_ntff_profile_hook = get_axon_ntff_profile_hook
    sys.modules["antenv.axon_hooks"] = mod
    import antenv
    antenv.axon_hooks = mod
    try:
        from trn_agent_boot.trn_boot import _ntff_profile_via_ctypes
        hook = _ntff_profile_via_ctypes("/opt/axon/libaxon_pjrt.so")
        if hook is not None:
            mod._hook = hook
    except Exception as e:  # degrade to no tracing
        print("ntff hook install failed:", e, file=sys.stderr)


def run_shards(cfg, inputs, trace=False):
    """Compile (cached), run on 8 cores, return (per-core outs, results)."""
    from concourse import bass_utils
    if trace:
        _install_ntff_hook()
    in_maps, gate_t, triv = make_in_maps(cfg, inputs)
    nc = _get_program(cfg, gate_t, triv)
    res = bass_utils.run_bass_kernel_spmd(
        nc, in_maps, core_ids=list(range(NCORES)), trace=trace)
    return [r["out"] for r in res.results], res


def kernel(**inputs):
    outs, _ = run_shards(FULL_CFG, inputs, trace=False)
    SQ = FULL_CFG["SQ"]
    full = np.empty((B, S, HID), np.float32)
    for c in range(NCORES):
        b_, half = divmod(c, 2)
        full[b_, half * SQ:(half + 1) * SQ, :] = outs[c]
    return full


# revision 62
# speedup vs baseline: 1.1423x; 1.0002x over previous
"""Trainium2 Bass kernel for nn_Attention_58360015618558 (v2).

Strategy (8 NeuronCores, SPMD):
  - Shard: core c -> (batch b = c//2, seq-half h = c%2); each core produces
    the output rows for its 1024 query positions.
  - K/V computed for the local seq-half only, AllGathered within the
    (even, odd) core pair; collectives overlap Q/caption projections.
  - LayerNorm mean folded into host-centered weights (z - mean(z) =
    x @ w_centered); LN rsqrt fused into the RoPE cos/sin multipliers, so
    the projection epilogue is 2 DVE passes + 2 gpsimd strided passes.
  - Attention in transposed-score layout (S^T[k,q]); exp on ACT over
    [128,1024] two-bank PSUM tiles; softmax denominator via fp8e5 (e5m2)
    DoubleRow matmuls (2x PE throughput; positive sums average the fp8
    quantization error down to ~0.2%).
  - K^T, V, Kc^T, Vc, and the attention output a^T all SBUF-resident;
    wo resident during phase B; single batched DMAs with multi-dim APs
    everywhere (DMA prepare costs ~1us of engine time per trigger).
"""

import math
import sys

import numpy as np

sys.path.insert(0, "/opt/trn_rl_repo")

import ml_dtypes  # noqa: E402

BF16 = ml_dtypes.bfloat16

# Full-size problem config
HID, H, KV, D, CAP = 2048, 16, 8, 128, 2048
B, S, LC = 4, 2048, 256
EPS = 1e-5
NCORES = 8

FULL_CFG = dict(S=S, SQ=S // 2, HID=HID, CAP=CAP, LC=LC, H=H, KV=KV)

# fp8e5 DoubleRow den: numerically wrong on HW (DoubleRow operand layout
# differs from the interp model) and no faster than bf16 -- keep off.
DEN_FP8 = False
EXP_PAIR = True   # single exp over [128,1024] two-bank PSUM tiles


def _build(cfg, gate_t, ln_trivial=(True, True, True)):
    """Build the per-core Bass program. Returns compiled Bacc."""
    import concourse.bass as bass  # noqa: F401
    import concourse.mybir as mybir
    import concourse.tile as tile
    from concourse import bacc
    from concourse.masks import make_identity
    from contextlib import ExitStack

    FP = mybir.dt.float32
    BF = mybir.dt.bfloat16
    F8 = mybir.dt.float8e5
    AF = mybir.ActivationFunctionType
    ALU = mybir.AluOpType
    DR = mybir.MatmulPerfMode.DoubleRow

    S_, SQ, HID_, CAP_, LC_ = cfg["S"], cfg["SQ"], cfg["HID"], cfg["CAP"], cfg["LC"]
    H_, KV_ = cfg["H"], cfg["KV"]
    HD, KD = H_ * D, KV_ * D
    CT, CTC = HID_ // 128, CAP_ // 128   # contraction tiles for x / caption
    NQ, NK, NLC = SQ // 128, S_ // 128, LC_ // 128
    SCALE = 1.0 / math.sqrt(D)
    qtriv, ktriv, kctriv = ln_trivial

    nc = bacc.Bacc("TRN2", target_bir_lowering=False, debug=False,
                   num_devices=NCORES)

    def din(name, shape, dt=BF):
        return nc.dram_tensor(name, shape, dt, kind="ExternalInput").ap()

    xTq = din("xTq", [HID_, SQ])        # x[b].T q-half columns (per core)
    capT = din("capT", [CAP_, LC_])
    wq = din("wq", [HID_, HD])          # column-centered host-side
    wk = din("wk", [HID_, KD])          # column-centered host-side
    wv = din("wv", [HID_, KD])
    wkc = din("wkc", [CAP_, KD])        # column-centered host-side
    wvc = din("wvc", [CAP_, KD])
    wo = din("wo", [HD, HID_])
    cosq = din("cosq", [SQ, D])   # interleaved: c[s,2i]=c[s,2i+1]=cos[s,i]
    sinq = din("sinq", [SQ, D])   # interleaved signed: -sin even, +sin odd
    lnw = {}
    for nm, dflat in (("q", HD), ("k", KD), ("kc", KD)):
        lnw[nm] = (din(f"ln_{nm}_w", [dflat], FP), din(f"ln_{nm}_b", [dflat], FP))
    out = nc.dram_tensor("out", [SQ, HID_], FP, kind="ExternalOutput").ap()

    # SPMD: all 8 cores run one graph.  Local K/V halves are staged to
    # _loc DRAM, AllGathered within the core pair, and both halves are
    # re-filled from the gathered tensor (rank-independent).  Collective
    # completion is enforced by gpsimd queue order (fills on gpsimd).
    with ExitStack() as top:
        tc = top.enter_context(tile.TileContext(nc))

        constp = top.enter_context(tc.tile_pool(name="const", bufs=1))
        resp = top.enter_context(tc.tile_pool(name="res", bufs=1))
        dramp = top.enter_context(tc.tile_pool(name="dram", bufs=1, space="DRAM"))

        ident = constp.tile([128, 128], BF, tag="ident", name="ident")
        make_identity(nc, ident[:])
        ones8 = constp.tile([128, 2, 128], F8, tag="ones8", name="ones8")
        nc.vector.memset(ones8[:], 1.0)
        ones_bk = constp.tile([128, 128], BF, tag="ones_bk", name="ones_bk")
        nc.vector.memset(ones_bk[:], 1.0)
        zero_c = constp.tile([128, 1], FP, tag="zero_c", name="zero_c")
        nc.vector.memset(zero_c[:], 0.0)
        nc.const_aps.aps[(FP, 0.0)] = zero_c[:]
        for ci, v in enumerate({float(KD * EPS), float(HD * EPS)}):
            ec_ = constp.tile([128, 1], FP, tag=f"eps{ci}", name=f"eps{ci}")
            nc.vector.memset(ec_[:], v)
            nc.const_aps.aps[(FP, v)] = ec_[:]

        # LN affine params (only loaded when nontrivial)
        affs = {}
        for nm, dflat, triv in (("q", HD, qtriv), ("k", KD, ktriv),
                                ("kc", KD, kctriv)):
            if not triv:
                wsb = constp.tile([128, dflat // 128], FP, tag=f"aw_{nm}",
                                  name=f"aw_{nm}")
                bsb = constp.tile([128, dflat // 128], FP, tag=f"ab_{nm}",
                                  name=f"ab_{nm}")
                nc.gpsimd.dma_start(wsb[:], lnw[nm][0].rearrange(
                    "(o p) -> p o", p=128))
                nc.gpsimd.dma_start(bsb[:], lnw[nm][1].rearrange(
                    "(o p) -> p o", p=128))
                affs[nm] = (wsb, bsb)

        # SBUF residents (V_res's pool opens later, after wq frees)
        KT_sb = resp.tile([128, KV_, 2, SQ], BF, tag="KT_sb", name="KT_sb")
        KcT_sb = resp.tile([128, KV_, LC_], BF, tag="KcT_sb", name="KcT_sb")
        Vc_res = resp.tile([128, NLC, KD], BF, tag="Vc_res", name="Vc_res")

        # DRAM intermediates
        KT_loc = dramp.tile([KV_, 128, SQ], BF, tag="KT_loc", name="KT_loc")
        KT_g = dramp.tile([2, KV_, 128, SQ], BF, tag="KT_g", name="KT_g")
        V_loc = dramp.tile([SQ, KD], BF, tag="V_loc", name="V_loc")
        V_g = dramp.tile([2, SQ, KD], BF, tag="V_g", name="V_g")
        QT = dramp.tile([128, H_, SQ], BF, tag="QT", name="QT")  # [d, h, s]

        # ---------------- projection pass ----------------
        def proj(ctx, xt_tiles, n_ct, w_tiles, dflat, n_st, ln, rope,
                 cos_sb, sin_sb, tgt, aff, swap_eng=None, lag=1,
                 ps_bufs=None):
            """One projection with fused LN(+RoPE) epilogue.

            xt_tiles/w_tiles: list of SBUF tiles [128, g, *] covering n_ct
              contraction tiles (group size g).
            tgt: ("kt", kt_dst_fn)   hh -> AP to write [128,128] transposed
                 ("qt", None)        -> stage [128,H,128] + 1 DMA to QT
                 ("vres", slot_ap_fn) st, i, W -> AP for plain copy
            lag: epilogue runs `lag` st-iterations behind the matmuls
              (needs ps_bufs >= lag+1 to avoid PE stalls).
            """
            W = 512
            NCH = dflat // W
            gszx = n_ct // len(xt_tiles)
            gszw = n_ct // len(w_tiles)
            if ps_bufs is None:
                ps_bufs = [2] * NCH
            sbufs = lag + 1
            psp = ctx.enter_context(tc.tile_pool(name="pjps", bufs=1,
                                                 space="PSUM"))
            tpp = ctx.enter_context(tc.tile_pool(name="tp", bufs=2,
                                                 space="PSUM"))
            sc = ctx.enter_context(tc.tile_pool(name="pjsc", bufs=2))
            qsp = ctx.enter_context(tc.tile_pool(name="qstg", bufs=2))

            SQRTD = math.sqrt(float(dflat))

            def epilogue(ps, st):
                nh = dflat // 128
                if ln:
                    # variance of the (already mean-centered) projection
                    stats = sc.tile([128, 8], FP, tag="stats", name="stats")
                    sqj = sc.tile([128, W], FP, tag="sqj", name="sqj")
                    for i in range(NCH):
                        nc.scalar.activation(sqj[:], ps[i][:], AF.Square,
                                             accum_out=stats[:, i:i + 1])
                    width = 1
                    while width < NCH:
                        for i in range(0, NCH, 2 * width):
                            if i + width < NCH:
                                nc.vector.tensor_tensor(
                                    stats[:, i:i + 1], stats[:, i:i + 1],
                                    stats[:, i + width:i + width + 1], ALU.add)
                        width *= 2
                    nc.scalar.activation(stats[:, 6:7], stats[:, 0:1],
                                         AF.Sqrt, bias=float(dflat * EPS))
                    rs = stats[:, 7:8]
                    nc.vector.reciprocal(rs, stats[:, 6:7])

                roped = sc.tile([128, dflat], BF, tag="roped", name="roped",
                                bufs=sbufs)
                if rope:
                    # o = z*(rs*c*sqrt(d)) + swap(z)*(rs*s_signed*sqrt(d))
                    rc = sc.tile([128, D], FP, tag="rc", name="rc")
                    rsig = sc.tile([128, D], FP, tag="rsig", name="rsig")
                    nc.vector.tensor_scalar(rc[:], cos_sb[:, st, :], rs,
                                            SQRTD, ALU.mult, ALU.mult)
                    nc.vector.tensor_scalar(rsig[:], sin_sb[:, st, :], rs,
                                            SQRTD, ALU.mult, ALU.mult)
                    rce = rc[:, None, :]
                    sw = sc.tile([128, dflat], BF, tag="sw", name="sw",
                                 bufs=sbufs)
                    rsig_v = rsig.rearrange("p (i two) -> p i two", two=2)
                    for i in range(NCH):
                        wh = W // D
                        rv = roped[:, i * W:(i + 1) * W].rearrange(
                            "p (h d) -> p h d", h=wh)
                        pv = ps[i].rearrange("p (h d) -> p h d", h=wh)
                        nc.vector.tensor_tensor(
                            rv, pv, rce.to_broadcast([128, wh, D]), ALU.mult)
                        p2 = ps[i].rearrange("p (h i two) -> p h i two",
                                             two=2, h=wh)
                        s2 = sw[:, i * W:(i + 1) * W].rearrange(
                            "p (h i two) -> p h i two", two=2, h=wh)
                        swap_eng.tensor_tensor(
                            s2[:, :, :, 0], p2[:, :, :, 1],
                            rsig_v[:, None, :, 0].to_broadcast(
                                [128, wh, D // 2]), ALU.mult)
                        swap_eng.tensor_tensor(
                            s2[:, :, :, 1], p2[:, :, :, 0],
                            rsig_v[:, None, :, 1].to_broadcast(
                                [128, wh, D // 2]), ALU.mult)
                        nc.vector.tensor_tensor(
                            roped[:, i * W:(i + 1) * W],
                            roped[:, i * W:(i + 1) * W],
                            sw[:, i * W:(i + 1) * W], ALU.add)
                elif ln:
                    for i in range(NCH):
                        nc.vector.tensor_scalar(
                            roped[:, i * W:(i + 1) * W], ps[i][:], rs,
                            SQRTD, ALU.mult, ALU.mult)
                else:
                    mode, dst_fn = tgt
                    if mode == "vdram":
                        vstg = qsp.tile([128, dflat], BF, tag="vstg",
                                        name="vstg")
                        for i in range(NCH):
                            nc.scalar.copy(vstg[:, i * W:(i + 1) * W],
                                           ps[i][:])
                        nc.sync.dma_start(
                            dst_fn[st * 128:(st + 1) * 128, :], vstg[:])
                    else:
                        for i in range(NCH):
                            nc.scalar.copy(dst_fn(st, i, W), ps[i][:])
                    return

                # transpose + write out
                mode, dst_fn = tgt
                qstg = None
                if mode == "qt":
                    qstg = qsp.tile([128, nh, 128], BF, tag="qstg",
                                    name="qstg")
                for hh in range(nh):
                    pst = tpp.tile([128, 128], BF, tag="tp", name="tp")
                    nc.tensor.transpose(pst[:],
                                        roped[:, hh * 128:(hh + 1) * 128],
                                        ident[:])
                    if mode == "qt":
                        dst = qstg[:, hh, :]
                    else:
                        dst = dst_fn(hh, st)
                    if aff is None:
                        nc.scalar.copy(dst, pst[:])
                    else:
                        wsb, bsb = aff
                        nc.vector.tensor_scalar(dst, pst[:],
                                                wsb[:, hh:hh + 1],
                                                bsb[:, hh:hh + 1],
                                                ALU.mult, ALU.add)
                if mode == "qt":
                    nc.sync.dma_start(QT[:, :, st * 128:(st + 1) * 128],
                                      qstg[:])

            # matmuls with pipelined epilogue (`lag` sts behind)
            pending = []
            for st in range(n_st):
                ps = [psp.tile([128, W], FP, tag=f"c{i}", name=f"c{i}",
                               bufs=ps_bufs[i])
                      for i in range(NCH)]
                for ct in range(n_ct):
                    gx, jx = divmod(ct, gszx)
                    gw, jw = divmod(ct, gszw)
                    for i in range(NCH):
                        nc.tensor.matmul(
                            ps[i][:],
                            lhsT=xt_tiles[gx][:, jx, st * 128:(st + 1) * 128],
                            rhs=w_tiles[gw][:, jw, i * W:(i + 1) * W],
                            start=(ct == 0), stop=(ct == n_ct - 1),
                        )
                pending.append((ps, st))
                if len(pending) > lag:
                    epilogue(*pending.pop(0))
            for args in pending:
                epilogue(*args)

        RG = [[2 * i, 2 * i + 1] for i in range(NCORES // 2)]
        NGRP = 4  # contraction-tile groups for pipelined loads

        # collectives signal completion on semaphores so the (fast) HWDGE
        # queues can issue the SBUF fills; the gpsimd SWDGE prepare costs
        # ~18ns/descriptor (~37us for a 2048-descriptor fill).
        agk_sem = top.enter_context(nc.semaphore(name="agk_sem"))
        agv_sem = top.enter_context(nc.semaphore(name="agv_sem"))
        nc.scalar.sem_clear(agk_sem)
        nc.sync.sem_clear(agv_sem)

        def load_grouped(pool, dram, n_ct, dflat, tagp, engine=None,
                         ngrp=NGRP):
            """Load [n_ct*128, dflat] weights as ngrp grouped tiles."""
            eng = engine or nc.gpsimd
            gsz = n_ct // ngrp
            tiles = []
            view = dram.rearrange("(g j p) d -> g p j d", p=128, j=gsz)
            for g in range(ngrp):
                t = pool.tile([128, gsz, dflat], BF, tag=f"{tagp}{g}",
                              name=f"{tagp}{g}")
                eng.dma_start(t[:], view[g])
                tiles.append(t)
            return tiles

        # ---------------- Phase A ----------------
        # Order: K (collective earliest) -> Q (big, hides AG-K) -> V ->
        # caption.  wq loads issue at t=0 on the idle vector queue; xtq/wk
        # split 8 ways for a fast start.  V_res's pool opens after wq frees.
        with ExitStack() as pa:
            xtp = pa.enter_context(tc.tile_pool(name="xtq", bufs=1))
            csp = pa.enter_context(tc.tile_pool(name="cs", bufs=1))
            cos_sb = csp.tile([128, NQ, D], BF, tag="cos", name="cos_sb")
            sin_sb = csp.tile([128, NQ, D], BF, tag="sin", name="sin_sb")
            nc.sync.dma_start(cos_sb[:],
                              cosq.rearrange("(st p) d -> p st d", p=128))
            nc.sync.dma_start(sin_sb[:],
                              sinq.rearrange("(st p) d -> p st d", p=128))
            # xtq: 8 row-groups x 2 column-halves on two HWDGE queues, so
            # the first K matmuls (needing only st0's columns) start early
            gszx = CT // 8
            xview = xTq.rearrange("(g j p) s -> g p j s", p=128, j=gszx)
            xt_tiles = []
            for g in range(8):
                t = xtp.tile([128, gszx, SQ], BF, tag=f"xt{g}",
                             name=f"xt{g}")
                nc.sync.dma_start(t[:, :, 0:SQ // 2],
                                  xview[g][:, :, 0:SQ // 2])
                nc.scalar.dma_start(t[:, :, SQ // 2:SQ],
                                    xview[g][:, :, SQ // 2:SQ])
                xt_tiles.append(t)
            wq_stack = ExitStack()
            wqp = wq_stack.enter_context(tc.tile_pool(name="wqp", bufs=1))
            wq_tiles = load_grouped(wqp, wq, CT, HD, "wq", engine=nc.scalar)

            with ExitStack() as pk:
                wkp = pk.enter_context(tc.tile_pool(name="wkp", bufs=1))
                wk_tiles = load_grouped(wkp, wk, CT, KD, "wk", ngrp=8)

                # K projection -> KT_sb own half + KT_loc -> AllGather
                def kt_dst(hh, st):
                    return KT_sb[:, hh, 0, st * 128:(st + 1) * 128]

                with ExitStack() as ph:
                    proj(ph, xt_tiles, CT, wk_tiles, KD, NQ, ln=True,
                         rope=True, cos_sb=cos_sb, sin_sb=sin_sb,
                         tgt=("kt", kt_dst), aff=affs.get("k"),
                         swap_eng=nc.vector, lag=2, ps_bufs=[3, 3])
                nc.sync.dma_start(KT_loc.rearrange("k d s -> d k s"),
                                  KT_sb[:, :, 0, :])
                nc.gpsimd.collective_compute(
                    "AllGather", ALU.bypass, replica_groups=RG,
                    ins=[KT_loc.opt()], outs=[KT_g.opt()])
                # gpsimd queue order == collective completion; signal it
                nc.gpsimd.sem_inc(agk_sem, 1)
                # both halves refreshed from gathered (SPMD-safe); HWDGE
                # fills gated on the collective via agk_sem
                nc.scalar.wait_ge(agk_sem, 1)
                for t in range(2):
                    nc.scalar.dma_start(
                        KT_sb[:, :, t, :],
                        KT_g[t].rearrange("k d s -> d k s"))

            # Q projection (AG-K overlaps this; rope swaps on DVE so
            # nothing here queues behind the collectives on gpsimd)
            with ExitStack() as ph:
                proj(ph, xt_tiles, CT, wq_tiles, HD, NQ, ln=True,
                     rope=True, cos_sb=cos_sb, sin_sb=sin_sb,
                     tgt=("qt", None), aff=affs.get("q"),
                     swap_eng=nc.vector, lag=1, ps_bufs=[2, 2, 1, 1])
            wq_stack.close()

            with ExitStack() as pt2:
                # caption loads issued early (scalar queue) so the caption
                # projections don't stall after V
                ctp = pt2.enter_context(tc.tile_pool(name="ct", bufs=1))
                wvcp = pt2.enter_context(tc.tile_pool(name="wvcp", bufs=1))
                cap_tiles = load_grouped(ctp, capT, CTC, LC_, "cap",
                                         engine=nc.scalar)
                wvc_tiles = load_grouped(wvcp, wvc, CTC, KD, "wvc",
                                         engine=nc.scalar)

                # V projection -> V_loc rows (DRAM) -> AllGather
                with ExitStack() as pv:
                    wvp = pv.enter_context(tc.tile_pool(name="wvp", bufs=1))
                    wv_tiles = load_grouped(wvp, wv, CT, KD, "wv")
                    with ExitStack() as ph:
                        proj(ph, xt_tiles, CT, wv_tiles, KD, NQ, ln=False,
                             rope=False, cos_sb=None, sin_sb=None,
                             tgt=("vdram", V_loc), aff=None,
                             ps_bufs=[3, 3])
                    nc.gpsimd.collective_compute(
                        "AllGather", ALU.bypass, replica_groups=RG,
                        ins=[V_loc.opt()], outs=[V_g.opt()])
                    nc.gpsimd.sem_inc(agv_sem, 1)

                # caption projections (overlap the AG-V tail)
                wkcp = pt2.enter_context(tc.tile_pool(name="wkcp", bufs=1))
                wkc_tiles = load_grouped(wkcp, wkc, CTC, KD, "wkc",
                                         engine=nc.scalar)

                def vc_dst(st, i, Wl):
                    return Vc_res[:, st, i * Wl:(i + 1) * Wl]

                with ExitStack() as ph:
                    proj(ph, cap_tiles, CTC, wvc_tiles, KD, NLC, ln=False,
                         rope=False, cos_sb=None, sin_sb=None,
                         tgt=("vres", vc_dst), aff=None)

                def kct_dst(hh, st):
                    return KcT_sb[:, hh, st * 128:(st + 1) * 128]

                with ExitStack() as ph:
                    proj(ph, cap_tiles, CTC, wkc_tiles, KD, NLC, ln=True,
                         rope=False, cos_sb=None, sin_sb=None,
                         tgt=("kt", kct_dst), aff=affs.get("kc"),
                         lag=2, ps_bufs=[3, 3])

        # ---------------- Phase B: attention ----------------
        # V_res opens only now (phase-A pools closed) and fills from V_g.
        # Per-kv column fills on the sync HWDGE queue (gated on the AG-V
        # semaphore) so kv=0 is ready first, right as phase B needs it.
        vresp = top.enter_context(tc.tile_pool(name="vres", bufs=1))
        V_res = vresp.tile([128, NK, KD], BF, tag="V_res", name="V_res")
        V_g_r = V_g.rearrange("t (st p) d -> p (t st) d", p=128)

        def emit_v_fills():
            # called right after the first qtw prefetch: phase B's first
            # scores then aren't stuck behind this wait on the sync queue
            nc.sync.wait_ge(agv_sem, 1)
            for kvf in range(KV_):
                nc.sync.dma_start(
                    V_res[:, :, kvf * 128:(kvf + 1) * 128],
                    V_g_r[:, :, kvf * 128:(kvf + 1) * 128])

        QCH = 512
        NQC = SQ // QCH
        NPAIR = NK // 2
        aTp = top.enter_context(tc.tile_pool(name="aTp", bufs=1))
        aT_sb = aTp.tile([128, H_, SQ], BF, tag="aT", name="aT_sb")
        wop = top.enter_context(tc.tile_pool(name="wop", bufs=1))
        wo_res = wop.tile([128, H_, HID_], BF, tag="wo", name="wo")
        wo_view = wo.rearrange("(g j p) e -> g p j e", p=128, j=H_ // NGRP)
        wo_tiles_view = wo_res.rearrange("p (g j) e -> g p j e",
                                         g=NGRP)
        for g in range(NGRP):
            nc.gpsimd.dma_start(wo_tiles_view[g], wo_view[g])

        with ExitStack() as pb:
            qp = pb.enter_context(tc.tile_pool(name="qw", bufs=2))
            ptp = pb.enter_context(tc.tile_pool(name="pt", bufs=NPAIR + 1))
            pt8p = pb.enter_context(tc.tile_pool(name="pt8", bufs=3))
            scp = pb.enter_context(tc.tile_pool(name="sc2", bufs=2))
            ps_s = pb.enter_context(tc.tile_pool(name="ps_s", bufs=2,
                                                 space="PSUM"))
            ps_o = pb.enter_context(tc.tile_pool(name="ps_o", bufs=1,
                                                 space="PSUM"))
            ps_oc = pb.enter_context(tc.tile_pool(name="ps_oc", bufs=1,
                                                  space="PSUM"))
            ps_d = pb.enter_context(tc.tile_pool(name="ps_d", bufs=1,
                                                 space="PSUM"))
            ps_dc = pb.enter_context(tc.tile_pool(name="ps_dc", bufs=1,
                                                  space="PSUM"))

            for kv in range(KV_):
                for rep in range(H_ // KV_):
                    h = kv * (H_ // KV_) + rep
                    qtw = qp.tile([128, SQ], BF, tag="qtw", name="qtw")
                    nc.sync.dma_start(qtw[:], QT[:, h, :])
                    if h == 0:
                        emit_v_fills()
                    for ch in range(NQC):
                        qs = qtw[:, ch * QCH:(ch + 1) * QCH]
                        po = ps_o.tile([128, QCH], FP, tag="po", name="po")
                        poc = ps_oc.tile([128, QCH], FP, tag="poc",
                                         name="poc")
                        pden = ps_d.tile([128, QCH], FP, tag="pden",
                                         name="pden")
                        pdenc = ps_dc.tile([128, QCH], FP, tag="pdenc",
                                           name="pdenc")
                        pts = [None] * NPAIR
                        pt8s = [None] * NPAIR

                        def emit_scores(p):
                            psc = ps_s.tile([128, 2 * QCH], FP, tag="psc",
                                            name="psc")
                            for half in range(2):
                                kt = 2 * p + half
                                nc.tensor.matmul(
                                    psc[:, half * QCH:(half + 1) * QCH],
                                    lhsT=KT_sb[:, kv, kt // NQ,
                                               (kt % NQ) * 128:
                                               (kt % NQ + 1) * 128],
                                    rhs=qs, start=True, stop=True)
                            pt = ptp.tile([128, 2 * QCH], BF, tag="pt",
                                          name="pt")
                            if EXP_PAIR:
                                nc.scalar.activation(pt[:], psc[:], AF.Exp,
                                                     scale=SCALE)
                            else:
                                for half in range(2):
                                    sl = slice(half * QCH, (half + 1) * QCH)
                                    nc.scalar.activation(pt[:, sl],
                                                         psc[:, sl], AF.Exp,
                                                         scale=SCALE)
                            pts[p] = pt
                            if DEN_FP8:
                                pt8 = pt8p.tile([128, 2, QCH], F8,
                                                tag="pt8", name="pt8")
                                nc.vector.tensor_copy(
                                    pt8.rearrange("p two q -> p (two q)"),
                                    pt[:])
                                pt8s[p] = pt8

                        def emit_av(p):
                            pt = pts[p]
                            for half in range(2):
                                kt = 2 * p + half
                                nc.tensor.matmul(
                                    po[:],
                                    lhsT=V_res[:, kt,
                                               kv * 128:(kv + 1) * 128],
                                    rhs=pt[:, half * QCH:(half + 1) * QCH],
                                    start=(kt == 0), stop=(kt == NK - 1))

                        def emit_den(p):
                            if DEN_FP8:
                                pt8 = pt8s[p]
                                for qh in range(2):
                                    nc.tensor.matmul(
                                        pden[:, qh * 256:(qh + 1) * 256],
                                        lhsT=ones8[:],
                                        rhs=pt8[:, :, qh * 256:(qh + 1) * 256],
                                        start=(p == 0),
                                        stop=(p == NPAIR - 1),
                                        perf_mode=DR)
                            else:
                                pt = pts[p]
                                for half in range(2):
                                    kt = 2 * p + half
                                    nc.tensor.matmul(
                                        pden[:],
                                        lhsT=ones_bk[:],
                                        rhs=pt[:, half * QCH:(half + 1) * QCH],
                                        start=(kt == 0), stop=(kt == NK - 1))

                        # all scores first: the first AV then comes ~8us
                        # into each block, absorbing V_res fill latency at
                        # phase-B start and keeping PE fed
                        for p in range(NPAIR):
                            emit_scores(p)
                        for p in range(NPAIR):
                            emit_av(p)
                            emit_den(p)

                        # caption attention (one kt pair)
                        pscc = ps_s.tile([128, 2 * QCH], FP, tag="psc",
                                         name="pscc")
                        for half in range(NLC):
                            nc.tensor.matmul(
                                pscc[:, half * QCH:(half + 1) * QCH],
                                lhsT=KcT_sb[:, kv,
                                            half * 128:(half + 1) * 128],
                                rhs=qs, start=True, stop=True)
                        ptc = ptp.tile([128, 2 * QCH], BF, tag="pt",
                                       name="ptc")
                        if EXP_PAIR:
                            nc.scalar.activation(ptc[:], pscc[:], AF.Exp,
                                                 scale=SCALE)
                        else:
                            for half in range(2):
                                sl = slice(half * QCH, (half + 1) * QCH)
                                nc.scalar.activation(ptc[:, sl], pscc[:, sl],
                                                     AF.Exp, scale=SCALE)
                        if DEN_FP8:
                            pt8c = pt8p.tile([128, 2, QCH], F8, tag="pt8",
                                             name="pt8c")
                            nc.vector.tensor_copy(
                                pt8c.rearrange("p two q -> p (two q)"),
                                ptc[:])
                        for half in range(NLC):
                            nc.tensor.matmul(
                                poc[:],
                                lhsT=Vc_res[:, half,
                                            kv * 128:(kv + 1) * 128],
                                rhs=ptc[:, half * QCH:(half + 1) * QCH],
                                start=(half == 0), stop=(half == NLC - 1))
                        if DEN_FP8:
                            for qh in range(2):
                                nc.tensor.matmul(
                                    pdenc[:, qh * 256:(qh + 1) * 256],
                                    lhsT=ones8[:],
                                    rhs=pt8c[:, :, qh * 256:(qh + 1) * 256],
                                    start=True, stop=True, perf_mode=DR)
                        else:
                            for half in range(NLC):
                                nc.tensor.matmul(
                                    pdenc[:],
                                    lhsT=ones_bk[:],
                                    rhs=ptc[:, half * QCH:(half + 1) * QCH],
                                    start=(half == 0),
                                    stop=(half == NLC - 1))

                        # epilogue -> aT_sb
                        aslice = aT_sb[:, h, ch * QCH:(ch + 1) * QCH]
                        rden = scp.tile([128, QCH], FP, tag="rden",
                                        name="rden")
                        nc.vector.reciprocal_approx_fast(rden[:], pden[:])
                        rdenc = scp.tile([128, QCH], FP, tag="rdenc",
                                         name="rdenc")
                        nc.vector.reciprocal_approx_fast(rdenc[:], pdenc[:])
                        tmp = scp.tile([128, QCH], FP, tag="tmp", name="tmp")
                        nc.vector.scalar_tensor_tensor(
                            tmp[:], poc[:], float(gate_t[h]), rdenc[:],
                            ALU.mult, ALU.mult)
                        nc.vector.tensor_tensor(aslice, po[:], rden[:],
                                                ALU.mult)
                        nc.vector.tensor_tensor(aslice, aslice, tmp[:],
                                                ALU.add)

        # ---------------- Phase C: output projection ----------------
        with ExitStack() as pc:
            op_ = pc.enter_context(tc.tile_pool(name="osb", bufs=2))
            cps = pc.enter_context(tc.tile_pool(name="cps", bufs=2,
                                                space="PSUM"))
            EW = 512
            NEC = HID_ // EW
            for st in range(NQ):
                ostg = op_.tile([128, HID_], FP, tag="ostg", name="ostg")
                for ec in range(NEC):
                    pso = cps.tile([128, EW], FP, tag="cps", name="cps")
                    for hh in range(H_):
                        nc.tensor.matmul(
                            pso[:],
                            lhsT=aT_sb[:, hh, st * 128:(st + 1) * 128],
                            rhs=wo_res[:, hh, ec * EW:(ec + 1) * EW],
                            start=(hh == 0), stop=(hh == H_ - 1))
                    nc.scalar.copy(ostg[:, ec * EW:(ec + 1) * EW], pso[:])
                nc.sync.dma_start(out[st * 128:(st + 1) * 128, :], ostg[:])

    nc.compile()
    return nc


_CACHE = {}


def _get_program(cfg, gate_t, ln_trivial):
    key = (tuple(sorted(cfg.items())), tuple(np.round(gate_t, 8)), ln_trivial)
    if key not in _CACHE:
        _CACHE[key] = _build(cfg, gate_t, ln_trivial)
    return _CACHE[key]


def make_in_maps(cfg, inputs):
    """Host-side sharding: returns (in_maps, gate_t, ln_trivial)."""
    S_, SQ = cfg["S"], cfg["SQ"]
    x = np.asarray(inputs["x"], np.float32)
    cap = np.asarray(inputs["caption_feat"], np.float32)
    cos = np.ascontiguousarray(np.asarray(inputs["freqs_cos"], np.float32))
    sin = np.ascontiguousarray(np.asarray(inputs["freqs_sin"], np.float32))
    gate_t = np.tanh(np.asarray(inputs["gate"], np.float32))

    def bf(a):
        return np.ascontiguousarray(a).astype(BF16)

    def center(w):
        w = np.asarray(w, np.float32)
        return w - w.mean(axis=1, keepdims=True)

    weights = {
        "wq": bf(center(inputs["wq"])),
        "wk": bf(center(inputs["wk"])),
        "wv": bf(np.asarray(inputs["wv"], np.float32)),
        "wo": bf(np.asarray(inputs["wo"], np.float32)),
        "wkc": bf(center(inputs["wk_cap"])),
        "wvc": bf(np.asarray(inputs["wv_cap"], np.float32)),
    }

    lns = {}
    triv = []
    for nm, wk_, bk_ in (("q", "q_ln_w", "q_ln_b"), ("k", "k_ln_w", "k_ln_b"),
                         ("kc", "kc_ln_w", "kc_ln_b")):
        w = np.ascontiguousarray(np.asarray(inputs[wk_], np.float32))
        b = np.ascontiguousarray(np.asarray(inputs[bk_], np.float32))
        triv.append(bool(np.all(w == 1.0) and np.all(b == 0.0)))
        lns[f"ln_{nm}_w"] = w
        lns[f"ln_{nm}_b"] = b

    in_maps = []
    for c in range(NCORES):
        b_, half = divmod(c, 2)
        xTb = bf(x[b_].T)
        m = dict(
            xTq=np.ascontiguousarray(xTb[:, half * SQ:(half + 1) * SQ]),
            capT=bf(cap[b_].T),
            cosq=bf(np.repeat(cos[half * SQ:(half + 1) * SQ], 2, axis=1)),
            sinq=bf(np.repeat(sin[half * SQ:(half + 1) * SQ], 2, axis=1)
                    * np.tile([-1.0, 1.0], cos.shape[1]).astype(np.float32)),
            **weights, **lns,
        )
        in_maps.append(m)
    return in_maps, gate_t, tuple(triv)


def _install_ntff_hook():
    """Shim the missing antenv.axon_hooks module so trace=True can capture
    NTFF profiles via the axon .so (test-time only)."""
    import types

    try:
        import antenv.axon_hooks  # noqa: F401
        return
    except ImportError:
        pass
    mod = types.ModuleType("antenv.axon_hooks")
    mod._hook = None

    def set_axon_ntff_profile_hook(h):
        mod._hook = h

    def get_axon_ntff_profile_hook():
        return mod._hook

    mod.set_axon_ntff_profile_hook = set_axon_ntff_profile_hook
    mod.get_axon_ntff_profile_hook = get_axon_ntff_profile_hook
    sys.modules["antenv.axon_hooks"] = mod
    import antenv
    antenv.axon_hooks = mod
    try:
        from trn_agent_boot.trn_boot import _ntff_profile_via_ctypes
        hook = _ntff_profile_via_ctypes("/opt/axon/libaxon_pjrt.so")
        if hook is not None:
            mod._hook = hook
    except Exception as e:  # degrade to no tracing
        print("ntff hook install failed:", e, file=sys.stderr)


def run_shards(cfg, inputs, trace=False):
    """Compile (cached), run on 8 cores, return (per-core outs, results)."""
    from concourse import bass_utils
    if trace:
        _install_ntff_hook()
    in_maps, gate_t, triv = make_in_maps(cfg, inputs)
    nc = _get_program(cfg, gate_t, triv)
    res = bass_utils.run_bass_kernel_spmd(
        nc, in_maps, core_ids=list(range(NCORES)), trace=trace)
    return [r["out"] for r in res.results], res


def kernel(**inputs):
    outs, _ = run_shards(FULL_CFG, inputs, trace=False)
    SQ = FULL_CFG["SQ"]
    full = np.empty((B, S, HID), np.float32)
    for c in range(NCORES):
        b_, half = divmod(c, 2)
        full[b_, half * SQ:(half + 1) * SQ, :] = outs[c]
    return full
